# revision 11
# baseline (speedup 1.0000x reference)
"""Trainium2 Bass kernel for nn_AttentionLayer_67817533604501.

Per-sample gated multi-head attention:
  Q = einsum('blm,bhkm,hkm->blhk', queries, gate, Wq) + bq   (same for K, V)
  attn = softmax(Q K^T / sqrt(dk)); out = (attn V) @ Wo^T + bo
Returns (out, attn) like the reference.

Sharding: 8 cores = 4 batches x 2 head-groups (8 heads each). Replicated
weights are sliced per head-group on host; per-core partial outputs of the
final projection are summed on host (exact: bias terms bo and the V-bias
contribution Wo@bv are added on host, mathematically identical because
attention rows sum to 1).

Device design (per core), all in transposed layouts so no on-device input
transposes are needed:
  QT,KT [feat, seq]; V [seq, feat(+ones col)] ; scores^T [s, l]
  exp on ScalarE reading PSUM; column sums via the ones column of V fused
  into the AV matmul; PE transposes of E produce attn[l, s]; the 1/colsum
  normalization is fused into the PSUM->SBUF copies (split DVE/ACT).
  Matmuls run in float32r (full PE rate, ~11-bit mantissa).
  Software pipeline: head h's transposes+normalize are interleaved into
  head h+1's scores/exp window so the PE never waits on ScalarE.
"""
import numpy as np
from contextlib import ExitStack

import concourse.bass as bass
import concourse.mybir as mybir
import concourse.tile as tile
from concourse import bacc
from concourse.bass_utils import run_bass_kernel_spmd
from concourse.masks import make_identity
from concourse.tile import add_dep_helper

F32 = mybir.dt.float32
F32R = mybir.dt.float32r
AF = mybir.ActivationFunctionType

B, L_FULL, S_FULL = 4, 1024, 1024
D_MODEL, H, DK = 1024, 16, 64
N_CORES = 8


def build_core(DM=1024, L=1024, S=1024, HPC=8, DKc=64, reps=1):
    """Build the per-core Bass module: one batch sample, HPC heads."""
    F = HPC * DKc
    MT, FT, LT, ST = DM // 128, F // 128, L // 128, S // 128
    LC, SC, DC = L // 512, S // 512, DM // 512
    assert LC >= 1 and SC >= 1 and DC >= 1 and FT >= 1 and ST % 4 == 0

    nc = bacc.Bacc("TRN2", target_bir_lowering=False, debug=False)

    xqT = nc.dram_tensor("xqT", [DM, L], F32R, kind="ExternalInput")
    xkT = nc.dram_tensor("xkT", [DM, S], F32R, kind="ExternalInput")
    xvT = nc.dram_tensor("xvT", [DM, S], F32R, kind="ExternalInput")
    gT = nc.dram_tensor("gT", [DM, F], F32, kind="ExternalInput")
    wqT = nc.dram_tensor("wqT", [DM, F], F32, kind="ExternalInput")
    wkT = nc.dram_tensor("wkT", [DM, F], F32, kind="ExternalInput")
    wvT = nc.dram_tensor("wvT", [DM, F], F32, kind="ExternalInput")
    woT = nc.dram_tensor("woT", [F, DM], F32R, kind="ExternalInput")
    bq = nc.dram_tensor("bq", [128, FT], F32, kind="ExternalInput")
    bk = nc.dram_tensor("bk", [128, FT], F32, kind="ExternalInput")
    attn_d = nc.dram_tensor("attn", [HPC, L, S], F32, kind="ExternalOutput")
    outp_d = nc.dram_tensor("outp", [L, DM], F32, kind="ExternalOutput")

    cfg = dict(DM=DM, L=L, S=S, HPC=HPC, DKc=DKc, F=F, MT=MT, FT=FT, LT=LT,
               ST=ST, LC=LC, SC=SC, DC=DC, xqT=xqT, xkT=xkT, xvT=xvT, gT=gT,
               wqT=wqT, wkT=wkT, wvT=wvT, woT=woT, bq=bq, bk=bk,
               attn_d=attn_d, outp_d=outp_d)
    with tile.TileContext(nc) as tc:
        for rep in range(reps):
            if rep:
                tc.strict_bb_all_engine_barrier()
            _emit_body(nc, tc, cfg, rep)
    nc.finalize()
    return nc


def _emit_body(nc, tc, v, rep=0):
    DM, L, S, HPC, DKc = v["DM"], v["L"], v["S"], v["HPC"], v["DKc"]
    F, MT, FT, LT, ST = v["F"], v["MT"], v["FT"], v["LT"], v["ST"]
    LC, SC, DC = v["LC"], v["SC"], v["DC"]
    xqT, xkT, xvT = v["xqT"], v["xkT"], v["xvT"]
    gT, wqT, wkT, wvT, woT = v["gT"], v["wqT"], v["wkT"], v["wvT"], v["woT"]
    bq, bk, attn_d, outp_d = v["bq"], v["bk"], v["attn_d"], v["outp_d"]
    MH = MT // 2

    with ExitStack() as ctx:
        pers = ctx.enter_context(tc.tile_pool(name="pers", bufs=1))
        QT = pers.tile([128, FT, L], F32R, tag="QT")
        KT = pers.tile([128, FT, S], F32R, tag="KT")
        Vt = pers.tile([128, ST, HPC, DKc + 1], F32R, tag="Vt")
        AVT = pers.tile([128, FT, L], F32R, tag="AVT")
        wo_sb = pers.tile([128, FT, DM], F32R, tag="wo")
        bq_sb = pers.tile([128, FT], F32, tag="bq")
        bk_sb = pers.tile([128, FT], F32, tag="bk")
        idf = pers.tile([128, 128], F32, tag="idf")
        idr = pers.tile([128, 128], F32R, tag="idr")
        ones1 = pers.tile([128, 1], F32, tag="ones1")

        # ---------------- projections ----------------
        with ExitStack() as pctx:
            gpool = pctx.enter_context(tc.tile_pool(name="gpool", bufs=1))
            wpool = pctx.enter_context(tc.tile_pool(name="wpool", bufs=1))
            wepool = pctx.enter_context(tc.tile_pool(name="wepool", bufs=2))
            xpool = pctx.enter_context(tc.tile_pool(name="xpool", bufs=3))
            pj = pctx.enter_context(tc.tile_pool(name="pj", bufs=4, space="PSUM"))

            def chunkload(dst, dram2d, rr, nch):
                mc = MT // nch
                for i in range(nch):
                    nc.sync.dma_start(
                        dst[:, i * mc:(i + 1) * mc, :],
                        dram2d[i * (DM // nch):(i + 1) * (DM // nch), :]
                        .rearrange(rr, p=128))

            def load_xh(xdram, lc, name):
                xh = xpool.tile([128, MT, 512], F32R, tag="xh", name=name)
                for hh2 in range(2):
                    nc.gpsimd.dma_start(
                        out=xh[:, hh2 * MH:(hh2 + 1) * MH, :],
                        in_=xdram[hh2 * (DM // 2):(hh2 + 1) * (DM // 2),
                                  lc * 512:(lc + 1) * 512]
                        .rearrange("(mt p) l -> p mt l", p=128))
                return xh

            g_sb = gpool.tile([128, MT, F], F32, tag="g")

            def gate_weights(wdram, name, nch=2):
                w_sb = wpool.tile([128, MT, F], F32, tag="wraw", name=f"w_{name}")
                mc = MT // nch
                we = wepool.tile([128, MT, F], F32R, tag="weff", name=f"we_{name}")
                for i in range(nch):
                    nc.sync.dma_start(
                        w_sb[:, i * mc:(i + 1) * mc, :],
                        wdram[i * (DM // nch):(i + 1) * (DM // nch), :]
                        .rearrange("(mt p) f -> p mt f", p=128))
                    if name == "q":
                        # interleave gate chunk loads on the q (cold) path
                        nc.sync.dma_start(
                            g_sb[:, i * mc:(i + 1) * mc, :],
                            gT[i * (DM // nch):(i + 1) * (DM // nch), :]
                            .rearrange("(mt p) f -> p mt f", p=128))
                    sl = slice(i * mc, (i + 1) * mc)
                    nc.vector.tensor_mul(we[:, sl, :], w_sb[:, sl, :],
                                         g_sb[:, sl, :])
                return we

            def proj_qk(we, xhs, bias_sb, OT, pn):
                for lc, xh in enumerate(xhs):
                    for ft in range(FT):
                        ps = pj.tile([128, 512], F32, tag="pj",
                                     name=f"pj_{pn}{lc}_{ft}")
                        for mt in range(MT):
                            nc.tensor.matmul(
                                ps[:], we[:, mt, ft * 128:(ft + 1) * 128],
                                xh[:, mt, :],
                                start=(mt == 0), stop=(mt == MT - 1))
                        if (ft + lc) % 2 == 0:
                            nc.vector.tensor_scalar_add(
                                OT[:, ft, lc * 512:(lc + 1) * 512], ps[:],
                                bias_sb[:, ft:ft + 1])
                        else:
                            nc.scalar.activation(
                                OT[:, ft, lc * 512:(lc + 1) * 512], ps[:],
                                AF.Identity, bias=bias_sb[:, ft:ft + 1])

            # cold path: xq chunk 0 first, then quarter-chunked gate/wq
            xh_q = [load_xh(xqT, 0, "xh_q0")]
            we_q = gate_weights(wqT, "q", nch=4)
            xh_q.append(load_xh(xqT, 1, "xh_q1") if LC > 1 else None)
            if LC == 1:
                xh_q = xh_q[:1]
            nc.sync.dma_start(bq_sb[:], bq[:])
            nc.sync.dma_start(bk_sb[:], bk[:])
            make_identity(nc, idf[:])
            nc.vector.tensor_copy(idr[:], idf[:])
            nc.vector.memset(ones1[:], 1.0)

            we_k = gate_weights(wkT, "k")
            xh_k = [load_xh(xkT, 0, "xh_k0")]
            proj_qk(we_q, xh_q, bq_sb, QT, "q")
            if SC > 1:
                xh_k.append(load_xh(xkT, 1, "xh_k1"))
            we_v = gate_weights(wvT, "v")
            xh_v = [load_xh(xvT, 0, "xh_v0")]
            proj_qk(we_k, xh_k, bk_sb, KT, "k")
            if SC > 1:
                xh_v.append(load_xh(xvT, 1, "xh_v1"))
            nc.sync.dma_start(
                wo_sb[:], woT[:].rearrange("(ft p) d -> p ft d", p=128))

            for sc in range(SC):
                xh = xh_v[sc]
                for stl in range(4):
                    st = sc * 4 + stl
                    ps = pj.tile([128, F], F32, tag="pj", name=f"pj_v{st}")
                    for mt in range(MT):
                        nc.tensor.matmul(
                            ps[:], xh[:, mt, stl * 128:(stl + 1) * 128],
                            we_v[:, mt, 0:F],
                            start=(mt == 0), stop=(mt == MT - 1))
                    dst = Vt[:, st, :, 0:DKc]
                    src = ps[:].rearrange("p (h k) -> p h k", k=DKc)
                    if st % 2 == 0:
                        nc.vector.tensor_copy(dst, src)
                    else:
                        nc.scalar.copy(dst, src)
            nc.vector.tensor_copy(
                Vt[:, :, :, DKc:DKc + 1],
                ones1[:, None, None, :].to_broadcast([128, ST, HPC, 1]))

        # ---------------- attention (head-pipelined) ----------------
        with ExitStack() as actx:
            epool = actx.enter_context(tc.tile_pool(name="epool", bufs=2))
            scp = actx.enter_context(tc.tile_pool(name="scp", bufs=2, space="PSUM"))
            avp = actx.enter_context(tc.tile_pool(name="avp", bufs=1, space="PSUM"))
            tpp = actx.enter_context(tc.tile_pool(name="tpp", bufs=2, space="PSUM"))
            apool = actx.enter_context(tc.tile_pool(name="apool", bufs=3))
            ipool = actx.enter_context(tc.tile_pool(name="ipool", bufs=2))
            dpool = actx.enter_context(tc.tile_pool(name="dpool", bufs=1, space="DRAM"))

            inv_d = dpool.tile([HPC, L], F32, tag="invd")
            NG = ST // 4
            groups = [(lt, g2) for lt in range(LT) for g2 in range(NG)]
            per_st = max(1, len(groups) // ST)

            def emit_group(pend, gi):
                h0 = pend["h"]
                lt, g2 = groups[gi]
                E0, ppi0 = pend["E"], pend["ppi"]
                if g2 == 0:
                    pend["astg"] = apool.tile([128, S], F32, tag="astg",
                                              name=f"astg{rep}_{h0}_{lt}")
                astg = pend["astg"]
                tpt = tpp.tile([128, 512], F32R, tag="tp",
                               name=f"tp{rep}_{h0}_{lt}_{g2}")
                for q in range(4):
                    st = g2 * 4 + q
                    nc.tensor.matmul(
                        tpt[:, q * 128:(q + 1) * 128],
                        E0[:, st, lt * 128:(lt + 1) * 128],
                        idr[:], is_transpose=True,
                        start=(q == 0), stop=(q == 3))
                sl = astg[:, g2 * 512:(g2 + 1) * 512]
                if gi % 4 < 3:
                    nc.vector.tensor_scalar_mul(sl, tpt[:], ppi0[:, lt:lt + 1])
                else:
                    nc.scalar.activation(sl, tpt[:], AF.Copy,
                                         scale=ppi0[:, lt:lt + 1])
                if g2 == NG - 1:
                    nc.sync.dma_start(
                        attn_d[h0, lt * 128:(lt + 1) * 128, :], astg[:])

            pend = None
            lazy_norm = {}

            def avt_slice(h0):
                return AVT[(h0 % 2) * 64:(h0 % 2) * 64 + 64, h0 // 2, :]

            def emit_avt_norm(h0):
                rep_t0 = lazy_norm.pop(h0, None)
                if rep_t0 is not None:
                    sl0 = avt_slice(h0)
                    pq0 = (h0 % 2) * 64
                    nc.vector.tensor_mul(sl0, sl0, rep_t0[pq0:pq0 + 64, :])

            for h in range(HPC):
                emit_avt_norm(h - 2)
                E_h = epool.tile([128, ST, L], F32R, tag="E", name=f"E{rep}_{h}")
                pq = (h % 2) * 64
                fth = h // 2
                gi = 0
                for st in range(ST):
                    ps = scp.tile([128, L], F32, tag="sc",
                                  name=f"sc{rep}_{h}_{st}")
                    for c in range(LC):
                        nc.tensor.matmul(
                            ps[:, c * 512:(c + 1) * 512],
                            KT[pq:pq + 64, fth, st * 128:(st + 1) * 128],
                            QT[pq:pq + 64, fth, c * 512:(c + 1) * 512],
                            start=True, stop=True)
                    nc.scalar.activation(E_h[:, st, :], ps[:], AF.Exp)
                    if pend is not None:
                        for _ in range(per_st):
                            if gi < len(groups):
                                emit_group(pend, gi)
                                gi += 1
                if pend is not None:
                    while gi < len(groups):
                        emit_group(pend, gi)
                        gi += 1

                av = avp.tile([65, L], F32, tag="av", name=f"av{rep}_{h}")
                for c in range(LC):
                    for st in range(ST):
                        nc.tensor.matmul(
                            av[:, c * 512:(c + 1) * 512],
                            Vt[:, st, h, :],
                            E_h[:, st, c * 512:(c + 1) * 512],
                            start=(st == 0), stop=(st == ST - 1))
                inv_row = ipool.tile([1, L], F32, tag="invr",
                                     name=f"invr{rep}_{h}")
                nc.vector.reciprocal(inv_row[:], av[64:65, :])
                d_out = nc.gpsimd.dma_start(out=inv_d[h], in_=inv_row[:])
                ppi = ipool.tile([128, LT], F32, tag="invpp",
                                 name=f"invpp{rep}_{h}")
                d_pp = nc.gpsimd.dma_start(
                    out=ppi[:], in_=inv_d[h].rearrange("(t p) -> p t", p=128))
                add_dep_helper(d_pp.ins, d_out.ins, reason="inv bounce RAW pp")
                rep_t = ipool.tile([128, L], F32, tag="invrep",
                                   name=f"invrep{rep}_{h}")
                src = inv_d[h]
                rap = bass.AP(tensor=src.tensor, offset=src.offset,
                              ap=[[0, 64], [1, L]])
                d_rep = nc.gpsimd.dma_start(out=rep_t[pq:pq + 64, :], in_=rap)
                add_dep_helper(d_rep.ins, d_out.ins, reason="inv bounce RAW rep")
                # stash unnormalized AVT now (frees the av PSUM slot);
                # multiply by 1/colsum lazily, two heads later
                nc.vector.tensor_copy(avt_slice(h), av[0:64, :])
                lazy_norm[h] = rep_t
                pend = {"h": h, "E": E_h, "ppi": ppi, "astg": None}

            gi = 0
            while gi < len(groups):
                emit_group(pend, gi)
                gi += 1
            for h in (HPC - 2, HPC - 1):
                emit_avt_norm(h)

        # ---------------- output projection ----------------
        with ExitStack() as octx:
            opp = octx.enter_context(tc.tile_pool(name="opp", bufs=2, space="PSUM"))
            osb = octx.enter_context(tc.tile_pool(name="osb", bufs=3))
            for lt in range(LT):
                po = opp.tile([128, DM], F32, tag="op", name=f"op{rep}_{lt}")
                for dc in range(DC):
                    for jf in range(FT):
                        nc.tensor.matmul(
                            po[:, dc * 512:(dc + 1) * 512],
                            AVT[:, jf, lt * 128:(lt + 1) * 128],
                            wo_sb[:, jf, dc * 512:(dc + 1) * 512],
                            start=(jf == 0), stop=(jf == FT - 1))
                ob = osb.tile([128, DM], F32, tag="ob", name=f"ob{rep}_{lt}")
                if lt % 2:
                    nc.scalar.copy(ob[:], po[:])
                else:
                    nc.vector.tensor_copy(ob[:], po[:])
                nc.sync.dma_start(outp_d[lt * 128:(lt + 1) * 128, :], ob[:])


def make_in_maps(queries, keys, values, gate_scores, W_q, b_q, W_k, b_k,
                 W_v, W_o):
    """Host-side sharding + layout prep. Core c = b*2 + g."""
    HPC = H // 2
    F = HPC * DK
    in_maps = []
    c = np.ascontiguousarray
    for b in range(B):
        qT = c(queries[b].T)
        kT = c(keys[b].T)
        vT = c(values[b].T)
        for g in range(2):
            fsl = slice(g * F, (g + 1) * F)
            gTc = c(gate_scores[b, g * HPC:(g + 1) * HPC]
                    .reshape(F, D_MODEL).T)
            in_maps.append({
                "xqT": qT, "xkT": kT, "xvT": vT, "gT": gTc,
                "wqT": c((W_q[fsl] * 0.125).T),
                "wkT": c(W_k[fsl].T),
                "wvT": c(W_v[fsl].T),
                "woT": c(W_o[:, fsl].T),
                "bq": c((b_q[fsl] * 0.125).reshape(F // 128, 128).T),
                "bk": c(b_k[fsl].reshape(F // 128, 128).T),
            })
    return in_maps


_NC_CACHE = {}


def kernel(queries, keys, values, gate_scores, W_q, b_q, W_k, b_k, W_v, b_v,
           W_o, b_o):
    queries = np.asarray(queries, np.float32)
    keys = np.asarray(keys, np.float32)
    values = np.asarray(values, np.float32)
    gate_scores = np.asarray(gate_scores, np.float32)
    W_q, b_q = np.asarray(W_q, np.float32), np.asarray(b_q, np.float32)
    W_k, b_k = np.asarray(W_k, np.float32), np.asarray(b_k, np.float32)
    W_v, b_v = np.asarray(W_v, np.float32), np.asarray(b_v, np.float32)
    W_o, b_o = np.asarray(W_o, np.float32), np.asarray(b_o, np.float32)
    assert queries.shape == (B, L_FULL, D_MODEL)

    if "nc" not in _NC_CACHE:
        _NC_CACHE["nc"] = build_core()
    nc = _NC_CACHE["nc"]
    in_maps = make_in_maps(queries, keys, values, gate_scores, W_q, b_q,
                           W_k, b_k, W_v, W_o)
    res = run_bass_kernel_spmd(nc, in_maps, core_ids=list(range(N_CORES)))

    HPC = H // 2
    attn = np.empty((B, H, L_FULL, S_FULL), np.float32)
    out = np.empty((B, L_FULL, D_MODEL), np.float32)
    corr = (W_o @ b_v + b_o).astype(np.float32)
    for b in range(B):
        r0 = res.results[b * 2]
        r1 = res.results[b * 2 + 1]
        attn[b, 0:HPC] = r0["attn"]
        attn[b, HPC:H] = r1["attn"]
        out[b] = r0["outp"] + r1["outp"] + corr
    return out, attn


# revision 18
# speedup vs baseline: 1.4218x; 1.4218x over previous
"""Trainium2 Bass kernel for nn_AttentionLayer_67817533604501.

Per-sample gated multi-head attention:
  Q = einsum('blm,bhkm,hkm->blhk', queries, gate, Wq) + bq   (same for K, V)
  attn = softmax(Q K^T / sqrt(dk)); out = (attn V) @ Wo^T + bo
Returns (out, attn) like the reference.

Sharding: 8 cores = 4 batches x 2 head-groups (8 heads each). Replicated
weights are sliced per head-group on host; per-core partial outputs of the
final projection are summed on host (exact: bias terms bo and the V-bias
contribution Wo@bv are added on host, mathematically identical because
attention rows sum to 1).

Device design (per core), all in transposed layouts so no on-device input
transposes are needed:
  QT,KT [feat, seq]; V [seq, feat(+ones col)] ; scores^T [s, l]
  exp on ScalarE reading PSUM; column sums via the ones column of V fused
  into the AV matmul; PE transposes of E produce attn[l, s]; the 1/colsum
  normalization is fused into the PSUM->SBUF copies (split DVE/ACT).
  Matmuls run in float32r (full PE rate, ~11-bit mantissa).
  Software pipeline: head h's transposes+normalize are interleaved into
  head h+1's scores/exp window so the PE never waits on ScalarE.
"""
import numpy as np
from contextlib import ExitStack

import concourse.bass as bass
import concourse.mybir as mybir
import concourse.tile as tile
from concourse import bacc
from concourse.bass_utils import run_bass_kernel_spmd
from concourse.masks import make_identity
from concourse.tile import add_dep_helper

F32 = mybir.dt.float32
F32R = mybir.dt.float32r
AF = mybir.ActivationFunctionType

B, L_FULL, S_FULL = 4, 1024, 1024
D_MODEL, H, DK = 1024, 16, 64
N_CORES = 8


def build_core(DM=1024, L=1024, S=1024, HPC=8, DKc=64, reps=1):
    """Build the per-core Bass module: one batch sample, HPC heads."""
    F = HPC * DKc
    MT, FT, LT, ST = DM // 128, F // 128, L // 128, S // 128
    LC, SC, DC = L // 512, S // 512, DM // 512
    assert LC >= 1 and SC >= 1 and DC >= 1 and FT >= 1 and ST % 4 == 0

    nc = bacc.Bacc("TRN2", target_bir_lowering=False, debug=False)

    xqT = nc.dram_tensor("xqT", [DM, L], F32R, kind="ExternalInput")
    xkT = nc.dram_tensor("xkT", [DM, S], F32R, kind="ExternalInput")
    xvT = nc.dram_tensor("xvT", [DM, S], F32R, kind="ExternalInput")
    gT = nc.dram_tensor("gT", [DM, F], F32, kind="ExternalInput")
    wqT = nc.dram_tensor("wqT", [DM, F], F32, kind="ExternalInput")
    wkT = nc.dram_tensor("wkT", [DM, F], F32, kind="ExternalInput")
    wvT = nc.dram_tensor("wvT", [DM, F], F32, kind="ExternalInput")
    woT = nc.dram_tensor("woT", [F, DM], F32R, kind="ExternalInput")
    bq = nc.dram_tensor("bq", [128, FT], F32, kind="ExternalInput")
    bk = nc.dram_tensor("bk", [128, FT], F32, kind="ExternalInput")
    attn_d = nc.dram_tensor("attn", [HPC, L, S], F32, kind="ExternalOutput")
    outp_d = nc.dram_tensor("outp", [L, DM], F32, kind="ExternalOutput")

    cfg = dict(DM=DM, L=L, S=S, HPC=HPC, DKc=DKc, F=F, MT=MT, FT=FT, LT=LT,
               ST=ST, LC=LC, SC=SC, DC=DC, xqT=xqT, xkT=xkT, xvT=xvT, gT=gT,
               wqT=wqT, wkT=wkT, wvT=wvT, woT=woT, bq=bq, bk=bk,
               attn_d=attn_d, outp_d=outp_d)
    with tile.TileContext(nc) as tc:
        for rep in range(reps):
            if rep:
                tc.strict_bb_all_engine_barrier()
            _emit_body(nc, tc, cfg, rep)
    nc.finalize()
    return nc


def _emit_body(nc, tc, v, rep=0):
    DM, L, S, HPC, DKc = v["DM"], v["L"], v["S"], v["HPC"], v["DKc"]
    F, MT, FT, LT, ST = v["F"], v["MT"], v["FT"], v["LT"], v["ST"]
    LC, SC, DC = v["LC"], v["SC"], v["DC"]
    xqT, xkT, xvT = v["xqT"], v["xkT"], v["xvT"]
    gT, wqT, wkT, wvT, woT = v["gT"], v["wqT"], v["wkT"], v["wvT"], v["woT"]
    bq, bk, attn_d, outp_d = v["bq"], v["bk"], v["attn_d"], v["outp_d"]
    MH = MT // 2

    with ExitStack() as ctx:
        pers = ctx.enter_context(tc.tile_pool(name="pers", bufs=1))
        QT = pers.tile([128, FT, L], F32R, tag="QT")
        KT = pers.tile([128, FT, S], F32R, tag="KT")
        Vt = pers.tile([128, ST, HPC, DKc + 1], F32R, tag="Vt")
        AVT = pers.tile([128, FT, L], F32R, tag="AVT")
        wo_sb = pers.tile([128, FT, DM], F32R, tag="wo")
        bq_sb = pers.tile([128, FT], F32, tag="bq")
        bk_sb = pers.tile([128, FT], F32, tag="bk")
        idf = pers.tile([128, 128], F32, tag="idf")
        idr = pers.tile([128, 128], F32R, tag="idr")
        ones1 = pers.tile([128, 1], F32, tag="ones1")

        # ---------------- projections ----------------
        with ExitStack() as pctx:
            gpool = pctx.enter_context(tc.tile_pool(name="gpool", bufs=1))
            wpool = pctx.enter_context(tc.tile_pool(name="wpool", bufs=1))
            wepool = pctx.enter_context(tc.tile_pool(name="wepool", bufs=2))
            xpool = pctx.enter_context(tc.tile_pool(name="xpool", bufs=3))
            pj = pctx.enter_context(tc.tile_pool(name="pj", bufs=4, space="PSUM"))

            def load_xh(xdram, lc, name):
                xh = xpool.tile([128, MT, 512], F32R, tag="xh", name=name)
                for hh2 in range(2):
                    nc.gpsimd.dma_start(
                        out=xh[:, hh2 * MH:(hh2 + 1) * MH, :],
                        in_=xdram[hh2 * (DM // 2):(hh2 + 1) * (DM // 2),
                                  lc * 512:(lc + 1) * 512]
                        .rearrange("(mt p) l -> p mt l", p=128))
                return xh

            g_sb = gpool.tile([128, MT, F], F32, tag="g")

            def gate_weights(wdram, name, nch=2):
                w_sb = wpool.tile([128, MT, F], F32, tag="wraw", name=f"w_{name}")
                mc = MT // nch
                we = wepool.tile([128, MT, F], F32R, tag="weff",
                                 name=f"we_{name}")
                for i in range(nch):
                    nc.sync.dma_start(
                        w_sb[:, i * mc:(i + 1) * mc, :],
                        wdram[i * (DM // nch):(i + 1) * (DM // nch), :]
                        .rearrange("(mt p) f -> p mt f", p=128))
                    if name == "q":
                        # interleave gate chunk loads on the q (cold) path
                        nc.sync.dma_start(
                            g_sb[:, i * mc:(i + 1) * mc, :],
                            gT[i * (DM // nch):(i + 1) * (DM // nch), :]
                            .rearrange("(mt p) f -> p mt f", p=128))
                    sl = slice(i * mc, (i + 1) * mc)
                    nc.vector.tensor_mul(we[:, sl, :], w_sb[:, sl, :],
                                         g_sb[:, sl, :])
                return we

            def proj_qk(we, xhs, bias_sb, OT, pn):
                for lc, xh in enumerate(xhs):
                    for ft in range(FT):
                        ps = pj.tile([128, 512], F32, tag="pj",
                                     name=f"pj_{pn}{lc}_{ft}")
                        for mt in range(MT):
                            nc.tensor.matmul(
                                ps[:], we[:, mt, ft * 128:(ft + 1) * 128],
                                xh[:, mt, :],
                                start=(mt == 0), stop=(mt == MT - 1))
                        if (ft + lc) % 2 == 0:
                            nc.vector.tensor_scalar_add(
                                OT[:, ft, lc * 512:(lc + 1) * 512], ps[:],
                                bias_sb[:, ft:ft + 1])
                        else:
                            nc.scalar.activation(
                                OT[:, ft, lc * 512:(lc + 1) * 512], ps[:],
                                AF.Identity, bias=bias_sb[:, ft:ft + 1])

            # cold path: xq chunk 0 first, then quarter-chunked gate/wq
            xh_q = [load_xh(xqT, 0, "xh_q0")]
            we_q = gate_weights(wqT, "q", nch=4)
            if LC > 1:
                xh_q.append(load_xh(xqT, 1, "xh_q1"))
            nc.sync.dma_start(bq_sb[:], bq[:])
            nc.sync.dma_start(bk_sb[:], bk[:])
            make_identity(nc, idf[:])
            nc.vector.tensor_copy(idr[:], idf[:])
            nc.vector.memset(ones1[:], 1.0)

            we_k = gate_weights(wkT, "k")
            xh_k = [load_xh(xkT, 0, "xh_k0")]
            proj_qk(we_q, xh_q, bq_sb, QT, "q")
            if SC > 1:
                xh_k.append(load_xh(xkT, 1, "xh_k1"))
            we_v = gate_weights(wvT, "v")
            xh_v = [load_xh(xvT, 0, "xh_v0")]
            proj_qk(we_k, xh_k, bk_sb, KT, "k")
            if SC > 1:
                xh_v.append(load_xh(xvT, 1, "xh_v1"))
            nc.sync.dma_start(
                wo_sb[:], woT[:].rearrange("(ft p) d -> p ft d", p=128))

            for sc in range(SC):
                xh = xh_v[sc]
                for stl in range(4):
                    st = sc * 4 + stl
                    ps = pj.tile([128, F], F32, tag="pj", name=f"pj_v{st}")
                    for mt in range(MT):
                        nc.tensor.matmul(
                            ps[:], xh[:, mt, stl * 128:(stl + 1) * 128],
                            we_v[:, mt, 0:F],
                            start=(mt == 0), stop=(mt == MT - 1))
                    dst = Vt[:, st, :, 0:DKc]
                    src2 = ps[:].rearrange("p (h k) -> p h k", k=DKc)
                    if st % 2 == 0:
                        nc.vector.tensor_copy(dst, src2)
                    else:
                        nc.scalar.copy(dst, src2)
            nc.vector.tensor_copy(
                Vt[:, :, :, DKc:DKc + 1],
                ones1[:, None, None, :].to_broadcast([128, ST, HPC, 1]))

        # ---------------- attention (head-pipelined; V proj inside h0) ------
        with ExitStack() as actx:
            epool = actx.enter_context(tc.tile_pool(name="epool", bufs=2))
            scp = actx.enter_context(tc.tile_pool(name="scp", bufs=2, space="PSUM"))
            avp = actx.enter_context(tc.tile_pool(name="avp", bufs=1, space="PSUM"))
            tpp = actx.enter_context(tc.tile_pool(name="tpp", bufs=2, space="PSUM"))
            apool = actx.enter_context(tc.tile_pool(name="apool", bufs=4))
            ipool = actx.enter_context(tc.tile_pool(name="ipool", bufs=2))
            dpool = actx.enter_context(tc.tile_pool(name="dpool", bufs=1, space="DRAM"))

            inv_d = dpool.tile([HPC, L], F32, tag="invd")
            NG = ST // 4
            groups = [(lt, g2) for lt in range(LT) for g2 in range(NG)]
            per_st = max(1, len(groups) // ST)

            def emit_group(pend, gi):
                h0 = pend["h"]
                lt, g2 = groups[gi]
                E0, ppi0 = pend["E"], pend["ppi"]
                if g2 == 0:
                    pend["astg"] = apool.tile([128, S], F32, tag="astg",
                                              name=f"astg{rep}_{h0}_{lt}")
                astg = pend["astg"]
                tpt = tpp.tile([128, 512], F32R, tag="tp",
                               name=f"tp{rep}_{h0}_{lt}_{g2}")
                for q in range(4):
                    st = g2 * 4 + q
                    nc.tensor.matmul(
                        tpt[:, q * 128:(q + 1) * 128],
                        E0[:, st, lt * 128:(lt + 1) * 128],
                        idr[:], is_transpose=True,
                        start=(q == 0), stop=(q == 3))
                sl = astg[:, g2 * 512:(g2 + 1) * 512]
                if gi % 4 < 3:
                    nc.vector.tensor_scalar_mul(sl, tpt[:], ppi0[:, lt:lt + 1])
                else:
                    nc.scalar.activation(sl, tpt[:], AF.Copy,
                                         scale=ppi0[:, lt:lt + 1])
                if g2 == NG - 1:
                    nc.sync.dma_start(
                        attn_d[h0, lt * 128:(lt + 1) * 128, :], astg[:])

            pend = None
            lazy_norm = {}
            _head_seen = []

            def avt_slice(h0):
                return AVT[(h0 % 2) * 64:(h0 % 2) * 64 + 64, h0 // 2, :]

            def emit_avt_norm(h0):
                rep_t0 = lazy_norm.pop(h0, None)
                if rep_t0 is not None:
                    sl0 = avt_slice(h0)
                    pq0 = (h0 % 2) * 64
                    nc.vector.tensor_mul(sl0, sl0, rep_t0[pq0:pq0 + 64, :])

            for h in range(HPC):
                emit_avt_norm(h - 2)
                E_h = epool.tile([128, ST, L], F32R, tag="E", name=f"E{rep}_{h}")
                pq = (h % 2) * 64
                fth = h // 2
                gi = 0
                for st in range(ST):
                    ps = scp.tile([128, L], F32, tag="sc",
                                  name=f"sc{rep}_{h}_{st}")
                    for c in range(LC):
                        nc.tensor.matmul(
                            ps[:, c * 512:(c + 1) * 512],
                            KT[pq:pq + 64, fth, st * 128:(st + 1) * 128],
                            QT[pq:pq + 64, fth, c * 512:(c + 1) * 512],
                            start=True, stop=True)
                    nc.scalar.activation(E_h[:, st, :], ps[:], AF.Exp)
                    if pend is not None:
                        for _ in range(per_st):
                            if gi < len(groups):
                                emit_group(pend, gi)
                                gi += 1
                if pend is not None:
                    while gi < len(groups):
                        emit_group(pend, gi)
                        gi += 1

                av = avp.tile([65, L], F32, tag="av", name=f"av{rep}_{h}")
                for c in range(LC):
                    for st in range(ST):
                        nc.tensor.matmul(
                            av[:, c * 512:(c + 1) * 512],
                            Vt[:, st, h, :],
                            E_h[:, st, c * 512:(c + 1) * 512],
                            start=(st == 0), stop=(st == ST - 1))
                inv_row = ipool.tile([1, L], F32, tag="invr",
                                     name=f"invr{rep}_{h}")
                nc.vector.reciprocal(inv_row[:], av[64:65, :])
                d_out = nc.gpsimd.dma_start(out=inv_d[h], in_=inv_row[:])
                ppi = ipool.tile([128, LT], F32, tag="invpp",
                                 name=f"invpp{rep}_{h}")
                d_pp = nc.gpsimd.dma_start(
                    out=ppi[:], in_=inv_d[h].rearrange("(t p) -> p t", p=128))
                add_dep_helper(d_pp.ins, d_out.ins, reason="inv bounce RAW pp")
                rep_t = ipool.tile([128, L], F32, tag="invrep",
                                   name=f"invrep{rep}_{h}")
                src = inv_d[h]
                rap = bass.AP(tensor=src.tensor, offset=src.offset,
                              ap=[[0, 64], [1, L]])
                d_rep = nc.gpsimd.dma_start(out=rep_t[pq:pq + 64, :], in_=rap)
                add_dep_helper(d_rep.ins, d_out.ins, reason="inv bounce RAW rep")
                # stash unnormalized AVT now (frees the av PSUM slot);
                # multiply by 1/colsum lazily, two heads later
                nc.vector.tensor_copy(avt_slice(h), av[0:64, :])
                lazy_norm[h] = rep_t
                pend = {"h": h, "E": E_h, "ppi": ppi, "astg": None}

            gi = 0
            while gi < len(groups):
                emit_group(pend, gi)
                gi += 1
            emit_avt_norm(HPC - 2)
            emit_avt_norm(HPC - 1)

        # ---------------- output projection ----------------
        with ExitStack() as octx:
            opp = octx.enter_context(tc.tile_pool(name="opp", bufs=2, space="PSUM"))
            osb = octx.enter_context(tc.tile_pool(name="osb", bufs=3))
            for lt in range(LT):
                po = opp.tile([128, DM], F32, tag="op", name=f"op{rep}_{lt}")
                for dc in range(DC):
                    for jf in range(FT):
                        nc.tensor.matmul(
                            po[:, dc * 512:(dc + 1) * 512],
                            AVT[:, jf, lt * 128:(lt + 1) * 128],
                            wo_sb[:, jf, dc * 512:(dc + 1) * 512],
                            start=(jf == 0), stop=(jf == FT - 1))
                ob = osb.tile([128, DM], F32, tag="ob", name=f"ob{rep}_{lt}")
                if lt % 2:
                    nc.scalar.copy(ob[:], po[:])
                else:
                    nc.vector.tensor_copy(ob[:], po[:])
                nc.sync.dma_start(outp_d[lt * 128:(lt + 1) * 128, :], ob[:])


def make_in_maps(queries, keys, values, gate_scores, W_q, b_q, W_k, b_k,
                 W_v, W_o):
    """Host-side sharding + layout prep. Core c = b*2 + g."""
    HPC = H // 2
    F = HPC * DK
    in_maps = []
    c = np.ascontiguousarray
    for b in range(B):
        qT = c(queries[b].T)
        kT = c(keys[b].T)
        vT = c(values[b].T)
        for g in range(2):
            fsl = slice(g * F, (g + 1) * F)
            gTc = c(gate_scores[b, g * HPC:(g + 1) * HPC]
                    .reshape(F, D_MODEL).T)
            in_maps.append({
                "xqT": qT, "xkT": kT, "xvT": vT, "gT": gTc,
                "wqT": c((W_q[fsl] * 0.125).T),
                "wkT": c(W_k[fsl].T),
                "wvT": c(W_v[fsl].T),
                "woT": c(W_o[:, fsl].T),
                "bq": c((b_q[fsl] * 0.125).reshape(F // 128, 128).T),
                "bk": c(b_k[fsl].reshape(F // 128, 128).T),
            })
    return in_maps


_NC_CACHE = {}


def kernel(queries, keys, values, gate_scores, W_q, b_q, W_k, b_k, W_v, b_v,
           W_o, b_o):
    queries = np.asarray(queries, np.float32)
    keys = np.asarray(keys, np.float32)
    values = np.asarray(values, np.float32)
    gate_scores = np.asarray(gate_scores, np.float32)
    W_q, b_q = np.asarray(W_q, np.float32), np.asarray(b_q, np.float32)
    W_k, b_k = np.asarray(W_k, np.float32), np.asarray(b_k, np.float32)
    W_v, b_v = np.asarray(W_v, np.float32), np.asarray(b_v, np.float32)
    W_o, b_o = np.asarray(W_o, np.float32), np.asarray(b_o, np.float32)
    assert queries.shape == (B, L_FULL, D_MODEL)

    if "nc" not in _NC_CACHE:
        _NC_CACHE["nc"] = build_core()
    nc = _NC_CACHE["nc"]
    in_maps = make_in_maps(queries, keys, values, gate_scores, W_q, b_q,
                           W_k, b_k, W_v, W_o)
    res = run_bass_kernel_spmd(nc, in_maps, core_ids=list(range(N_CORES)))

    HPC = H // 2
    attn = np.empty((B, H, L_FULL, S_FULL), np.float32)
    out = np.empty((B, L_FULL, D_MODEL), np.float32)
    corr = (W_o @ b_v + b_o).astype(np.float32)
    for b in range(B):
        r0 = res.results[b * 2]
        r1 = res.results[b * 2 + 1]
        attn[b, 0:HPC] = r0["attn"]
        attn[b, HPC:H] = r1["attn"]
        out[b] = r0["outp"] + r1["outp"] + corr
    return out, attn


# revision 24
# speedup vs baseline: 1.4629x; 1.0289x over previous
"""Trainium2 Bass kernel for nn_AttentionLayer_67817533604501.

Per-sample gated multi-head attention:
  Q = einsum('blm,bhkm,hkm->blhk', queries, gate, Wq) + bq   (same for K, V)
  attn = softmax(Q K^T / sqrt(dk)); out = (attn V) @ Wo^T + bo
Returns (out, attn) like the reference.

Sharding: 8 cores = 4 batches x 2 head-groups (8 heads each). Replicated
weights are sliced per head-group on host; per-core partial outputs of the
final projection are summed on host (exact: bias terms bo and the V-bias
contribution Wo@bv are added on host, mathematically identical because
attention rows sum to 1).

Device design (per core), all in transposed layouts so no on-device input
transposes are needed:
  QT,KT [feat, seq]; V [seq, feat(+ones col)] ; scores^T [s, l]
  exp on ScalarE reading PSUM; column sums via the ones column of V fused
  into the AV matmul; PE transposes of E produce attn[l, s]; the 1/colsum
  normalization is fused into the PSUM->SBUF copies (split DVE/ACT).
  Matmuls run in float32r (full PE rate, ~11-bit mantissa).
  Software pipeline: head h's transposes+normalize are interleaved into
  head h+1's scores/exp window so the PE never waits on ScalarE.
"""
import numpy as np
from contextlib import ExitStack

import concourse.bass as bass
import concourse.mybir as mybir
import concourse.tile as tile
from concourse import bacc
from concourse.bass_utils import run_bass_kernel_spmd
from concourse.masks import make_identity
from concourse.tile import add_dep_helper

F32 = mybir.dt.float32
F32R = mybir.dt.float32r
AF = mybir.ActivationFunctionType

B, L_FULL, S_FULL = 4, 1024, 1024
D_MODEL, H, DK = 1024, 16, 64
N_CORES = 8


def build_core(DM=1024, L=1024, S=1024, HPC=8, DKc=64, reps=1):
    """Build the per-core Bass module: one batch sample, HPC heads."""
    F = HPC * DKc
    MT, FT, LT, ST = DM // 128, F // 128, L // 128, S // 128
    LC, SC, DC = L // 512, S // 512, DM // 512
    assert LC >= 1 and SC >= 1 and DC >= 1 and FT >= 1 and ST % 4 == 0

    nc = bacc.Bacc("TRN2", target_bir_lowering=False, debug=False)

    xqT = nc.dram_tensor("xqT", [DM, L], F32R, kind="ExternalInput")
    xkT = nc.dram_tensor("xkT", [DM, S], F32R, kind="ExternalInput")
    xvT = nc.dram_tensor("xvT", [DM, S], F32R, kind="ExternalInput")
    gT = nc.dram_tensor("gT", [DM, F], F32, kind="ExternalInput")
    wqT = nc.dram_tensor("wqT", [DM, F], F32, kind="ExternalInput")
    wkT = nc.dram_tensor("wkT", [DM, F], F32, kind="ExternalInput")
    wvT = nc.dram_tensor("wvT", [DM, F], F32, kind="ExternalInput")
    woT = nc.dram_tensor("woT", [F, DM], F32R, kind="ExternalInput")
    bq = nc.dram_tensor("bq", [128, FT], F32, kind="ExternalInput")
    bk = nc.dram_tensor("bk", [128, FT], F32, kind="ExternalInput")
    attn_d = nc.dram_tensor("attn", [HPC, L, S], F32, kind="ExternalOutput")
    outp_d = nc.dram_tensor("outp", [L, DM], F32, kind="ExternalOutput")

    cfg = dict(DM=DM, L=L, S=S, HPC=HPC, DKc=DKc, F=F, MT=MT, FT=FT, LT=LT,
               ST=ST, LC=LC, SC=SC, DC=DC, xqT=xqT, xkT=xkT, xvT=xvT, gT=gT,
               wqT=wqT, wkT=wkT, wvT=wvT, woT=woT, bq=bq, bk=bk,
               attn_d=attn_d, outp_d=outp_d)
    with tile.TileContext(nc) as tc:
        for rep in range(reps):
            if rep:
                tc.strict_bb_all_engine_barrier()
            _emit_body(nc, tc, cfg, rep)
    nc.finalize()
    return nc


def _emit_body(nc, tc, v, rep=0):
    DM, L, S, HPC, DKc = v["DM"], v["L"], v["S"], v["HPC"], v["DKc"]
    F, MT, FT, LT, ST = v["F"], v["MT"], v["FT"], v["LT"], v["ST"]
    LC, SC, DC = v["LC"], v["SC"], v["DC"]
    xqT, xkT, xvT = v["xqT"], v["xkT"], v["xvT"]
    gT, wqT, wkT, wvT, woT = v["gT"], v["wqT"], v["wkT"], v["wvT"], v["woT"]
    bq, bk, attn_d, outp_d = v["bq"], v["bk"], v["attn_d"], v["outp_d"]
    MH = MT // 2

    with ExitStack() as ctx:
        pers = ctx.enter_context(tc.tile_pool(name="pers", bufs=1))
        QT = pers.tile([128, FT, L], F32R, tag="QT")
        KT = pers.tile([128, FT, S], F32R, tag="KT")
        Vt = pers.tile([128, ST, HPC, DKc + 1], F32R, tag="Vt")
        AVT = pers.tile([128, FT, L], F32R, tag="AVT")
        wo_sb = pers.tile([128, FT, DM], F32R, tag="wo")
        bq_sb = pers.tile([128, FT], F32, tag="bq")
        bk_sb = pers.tile([128, FT], F32, tag="bk")
        warm1 = pers.tile([128, 1], F32, tag="warm1")
        idf = pers.tile([128, 128], F32, tag="idf")
        idr = pers.tile([128, 128], F32R, tag="idr")
        ones1 = pers.tile([128, 1], F32, tag="ones1")

        # ---------------- projections ----------------
        with ExitStack() as pctx:
            gpool = pctx.enter_context(tc.tile_pool(name="gpool", bufs=1))
            wpool = pctx.enter_context(tc.tile_pool(name="wpool", bufs=1))
            wepool = pctx.enter_context(tc.tile_pool(name="wepool", bufs=2))
            xpool = pctx.enter_context(tc.tile_pool(name="xpool", bufs=3))
            pj = pctx.enter_context(tc.tile_pool(name="pj", bufs=4, space="PSUM"))

            def load_xh(xdram, lc, name, nch=2):
                xh = xpool.tile([128, MT, 512], F32R, tag="xh", name=name)
                mc = MT // nch
                for i in range(nch):
                    nc.gpsimd.dma_start(
                        out=xh[:, i * mc:(i + 1) * mc, :],
                        in_=xdram[i * (DM // nch):(i + 1) * (DM // nch),
                                  lc * 512:(lc + 1) * 512]
                        .rearrange("(mt p) l -> p mt l", p=128))
                return xh

            g_sb = gpool.tile([128, MT, F], F32, tag="g")

            def gate_weights(wdram, name, nch=2):
                w_sb = wpool.tile([128, MT, F], F32, tag="wraw", name=f"w_{name}")
                mc = MT // nch
                we = wepool.tile([128, MT, F], F32R, tag="weff",
                                 name=f"we_{name}")
                for i in range(nch):
                    nc.sync.dma_start(
                        w_sb[:, i * mc:(i + 1) * mc, :],
                        wdram[i * (DM // nch):(i + 1) * (DM // nch), :]
                        .rearrange("(mt p) f -> p mt f", p=128))
                    if name == "q":
                        # interleave gate chunk loads on the q (cold) path
                        nc.sync.dma_start(
                            g_sb[:, i * mc:(i + 1) * mc, :],
                            gT[i * (DM // nch):(i + 1) * (DM // nch), :]
                            .rearrange("(mt p) f -> p mt f", p=128))
                    sl = slice(i * mc, (i + 1) * mc)
                    nc.vector.tensor_mul(we[:, sl, :], w_sb[:, sl, :],
                                         g_sb[:, sl, :])
                return we

            def proj_qk(we, xhs, bias_sb, OT, pn):
                for lc, xh in enumerate(xhs):
                    for ft in range(FT):
                        ps = pj.tile([128, 512], F32, tag="pj",
                                     name=f"pj_{pn}{lc}_{ft}")
                        for mt in range(MT):
                            nc.tensor.matmul(
                                ps[:], we[:, mt, ft * 128:(ft + 1) * 128],
                                xh[:, mt, :],
                                start=(mt == 0), stop=(mt == MT - 1))
                        if (ft + lc) % 2 == 0:
                            nc.vector.tensor_scalar_add(
                                OT[:, ft, lc * 512:(lc + 1) * 512], ps[:],
                                bias_sb[:, ft:ft + 1])
                        else:
                            nc.scalar.activation(
                                OT[:, ft, lc * 512:(lc + 1) * 512], ps[:],
                                AF.Identity, bias=bias_sb[:, ft:ft + 1])

            # cold path: xq chunk 0 first, then quarter-chunked gate/wq
            xh_q = [load_xh(xqT, 0, "xh_q0")]
            we_q = gate_weights(wqT, "q", nch=4)
            if LC > 1:
                xh_q.append(load_xh(xqT, 1, "xh_q1"))
            nc.sync.dma_start(bq_sb[:], bq[:])
            nc.sync.dma_start(bk_sb[:], bk[:])
            make_identity(nc, idf[:])
            nc.vector.tensor_copy(idr[:], idf[:])
            nc.vector.memset(ones1[:], 1.0)
            # pin the exp table set early so the first real exp doesn't stall
            nc.scalar.activation(warm1[:], ones1[:], AF.Exp)

            we_k = gate_weights(wkT, "k")
            xh_k = [load_xh(xkT, 0, "xh_k0")]
            proj_qk(we_q, xh_q, bq_sb, QT, "q")
            if SC > 1:
                xh_k.append(load_xh(xkT, 1, "xh_k1"))
            we_v = gate_weights(wvT, "v")
            xh_v = [load_xh(xvT, 0, "xh_v0")]
            proj_qk(we_k, xh_k, bk_sb, KT, "k")
            if SC > 1:
                xh_v.append(load_xh(xvT, 1, "xh_v1"))
            nc.sync.dma_start(
                wo_sb[:], woT[:].rearrange("(ft p) d -> p ft d", p=128))

            for sc in range(SC):
                xh = xh_v[sc]
                for stl in range(4):
                    st = sc * 4 + stl
                    ps = pj.tile([128, F], F32, tag="pj", name=f"pj_v{st}")
                    for mt in range(MT):
                        nc.tensor.matmul(
                            ps[:], xh[:, mt, stl * 128:(stl + 1) * 128],
                            we_v[:, mt, 0:F],
                            start=(mt == 0), stop=(mt == MT - 1))
                    dst = Vt[:, st, :, 0:DKc]
                    src2 = ps[:].rearrange("p (h k) -> p h k", k=DKc)
                    if st % 2 == 0:
                        nc.vector.tensor_copy(dst, src2)
                    else:
                        nc.scalar.copy(dst, src2)
            nc.vector.tensor_copy(
                Vt[:, :, :, DKc:DKc + 1],
                ones1[:, None, None, :].to_broadcast([128, ST, HPC, 1]))

        # ---------------- attention (head-pipelined; V proj inside h0) ------
        with ExitStack() as actx:
            epool = actx.enter_context(tc.tile_pool(name="epool", bufs=3))
            scp = actx.enter_context(tc.tile_pool(name="scp", bufs=2, space="PSUM"))
            avp = actx.enter_context(tc.tile_pool(name="avp", bufs=1, space="PSUM"))
            tpp = actx.enter_context(tc.tile_pool(name="tpp", bufs=2, space="PSUM"))
            apool = actx.enter_context(tc.tile_pool(name="apool", bufs=3))
            ipool = actx.enter_context(tc.tile_pool(name="ipool", bufs=2))
            dpool = actx.enter_context(tc.tile_pool(name="dpool", bufs=1, space="DRAM"))

            inv_d = dpool.tile([HPC, L], F32, tag="invd")
            NG = ST // 4
            groups = [(lt, g2) for lt in range(LT) for g2 in range(NG)]
            per_st = max(1, len(groups) // ST)

            def emit_group(pend, gi):
                h0 = pend["h"]
                lt, g2 = groups[gi]
                E0, ppi0 = pend["E"], pend["ppi"]
                if g2 == 0:
                    pend["astg"] = apool.tile([128, S], F32, tag="astg",
                                              name=f"astg{rep}_{h0}_{lt}")
                astg = pend["astg"]
                tpt = tpp.tile([128, 512], F32R, tag="tp",
                               name=f"tp{rep}_{h0}_{lt}_{g2}")
                for q in range(4):
                    st = g2 * 4 + q
                    nc.tensor.matmul(
                        tpt[:, q * 128:(q + 1) * 128],
                        E0[:, st, lt * 128:(lt + 1) * 128],
                        idr[:], is_transpose=True,
                        start=(q == 0), stop=(q == 3))
                sl = astg[:, g2 * 512:(g2 + 1) * 512]
                if gi % 4 < 3:
                    nc.vector.tensor_scalar_mul(sl, tpt[:], ppi0[:, lt:lt + 1])
                else:
                    nc.scalar.activation(sl, tpt[:], AF.Copy,
                                         scale=ppi0[:, lt:lt + 1])
                if g2 == NG - 1:
                    nc.sync.dma_start(
                        attn_d[h0, lt * 128:(lt + 1) * 128, :], astg[:])

            pend = None
            lazy_norm = {}

            def avt_slice(h0):
                return AVT[(h0 % 2) * 64:(h0 % 2) * 64 + 64, h0 // 2, :]

            def emit_avt_norm(h0):
                rep_t0 = lazy_norm.pop(h0, None)
                if rep_t0 is not None:
                    sl0 = avt_slice(h0)
                    pq0 = (h0 % 2) * 64
                    nc.vector.tensor_mul(sl0, sl0, rep_t0[pq0:pq0 + 64, :])

            for h in range(HPC):
                emit_avt_norm(h - 2)
                E_h = epool.tile([128, ST, L], F32R, tag="E", name=f"E{rep}_{h}")
                pq = (h % 2) * 64
                fth = h // 2
                av = avp.tile([65, L], F32, tag="av", name=f"av{rep}_{h}")

                def emit_av(st0):
                    for c in range(LC):
                        nc.tensor.matmul(
                            av[:, c * 512:(c + 1) * 512],
                            Vt[:, st0, h, :],
                            E_h[:, st0, c * 512:(c + 1) * 512],
                            start=(st0 == 0), stop=(st0 == ST - 1))

                gi = 0
                for st in range(ST):
                    ps = scp.tile([128, L], F32, tag="sc",
                                  name=f"sc{rep}_{h}_{st}")
                    for c in range(LC):
                        nc.tensor.matmul(
                            ps[:, c * 512:(c + 1) * 512],
                            KT[pq:pq + 64, fth, st * 128:(st + 1) * 128],
                            QT[pq:pq + 64, fth, c * 512:(c + 1) * 512],
                            start=True, stop=True)
                    nc.scalar.activation(E_h[:, st, :], ps[:], AF.Exp)
                    if st > 0:
                        emit_av(st - 1)
                    if pend is not None:
                        for _ in range(per_st):
                            if gi < len(groups):
                                emit_group(pend, gi)
                                gi += 1
                if pend is not None:
                    while gi < len(groups):
                        emit_group(pend, gi)
                        gi += 1
                emit_av(ST - 1)
                inv_row = ipool.tile([1, L], F32, tag="invr",
                                     name=f"invr{rep}_{h}")
                nc.vector.reciprocal(inv_row[:], av[64:65, :])
                d_out = nc.gpsimd.dma_start(out=inv_d[h], in_=inv_row[:])
                ppi = ipool.tile([128, LT], F32, tag="invpp",
                                 name=f"invpp{rep}_{h}")
                d_pp = nc.gpsimd.dma_start(
                    out=ppi[:], in_=inv_d[h].rearrange("(t p) -> p t", p=128))
                add_dep_helper(d_pp.ins, d_out.ins, reason="inv bounce RAW pp")
                rep_t = ipool.tile([128, L], F32, tag="invrep",
                                   name=f"invrep{rep}_{h}")
                src = inv_d[h]
                rap = bass.AP(tensor=src.tensor, offset=src.offset,
                              ap=[[0, 64], [1, L]])
                d_rep = nc.gpsimd.dma_start(out=rep_t[pq:pq + 64, :], in_=rap)
                add_dep_helper(d_rep.ins, d_out.ins, reason="inv bounce RAW rep")
                # stash unnormalized AVT now (frees the av PSUM slot);
                # multiply by 1/colsum lazily, two heads later
                nc.vector.tensor_copy(avt_slice(h), av[0:64, :])
                lazy_norm[h] = rep_t
                pend = {"h": h, "E": E_h, "ppi": ppi, "astg": None}

            gi = 0
            while gi < len(groups):
                emit_group(pend, gi)
                gi += 1
            emit_avt_norm(HPC - 2)
            emit_avt_norm(HPC - 1)

        # ---------------- output projection ----------------
        with ExitStack() as octx:
            opp = octx.enter_context(tc.tile_pool(name="opp", bufs=2, space="PSUM"))
            osb = octx.enter_context(tc.tile_pool(name="osb", bufs=3))
            for lt in range(LT):
                po = opp.tile([128, DM], F32, tag="op", name=f"op{rep}_{lt}")
                for dc in range(DC):
                    for jf in range(FT):
                        nc.tensor.matmul(
                            po[:, dc * 512:(dc + 1) * 512],
                            AVT[:, jf, lt * 128:(lt + 1) * 128],
                            wo_sb[:, jf, dc * 512:(dc + 1) * 512],
                            start=(jf == 0), stop=(jf == FT - 1))
                ob = osb.tile([128, DM], F32, tag="ob", name=f"ob{rep}_{lt}")
                if lt % 2:
                    nc.scalar.copy(ob[:], po[:])
                else:
                    nc.vector.tensor_copy(ob[:], po[:])
                nc.sync.dma_start(outp_d[lt * 128:(lt + 1) * 128, :], ob[:])


def make_in_maps(queries, keys, values, gate_scores, W_q, b_q, W_k, b_k,
                 W_v, W_o):
    """Host-side sharding + layout prep. Core c = b*2 + g."""
    HPC = H // 2
    F = HPC * DK
    in_maps = []
    c = np.ascontiguousarray
    for b in range(B):
        qT = c(queries[b].T)
        kT = c(keys[b].T)
        vT = c(values[b].T)
        for g in range(2):
            fsl = slice(g * F, (g + 1) * F)
            gTc = c(gate_scores[b, g * HPC:(g + 1) * HPC]
                    .reshape(F, D_MODEL).T)
            in_maps.append({
                "xqT": qT, "xkT": kT, "xvT": vT, "gT": gTc,
                "wqT": c((W_q[fsl] * 0.125).T),
                "wkT": c(W_k[fsl].T),
                "wvT": c(W_v[fsl].T),
                "woT": c(W_o[:, fsl].T),
                "bq": c((b_q[fsl] * 0.125).reshape(F // 128, 128).T),
                "bk": c(b_k[fsl].reshape(F // 128, 128).T),
            })
    return in_maps


_NC_CACHE = {}


def kernel(queries, keys, values, gate_scores, W_q, b_q, W_k, b_k, W_v, b_v,
           W_o, b_o):
    queries = np.asarray(queries, np.float32)
    keys = np.asarray(keys, np.float32)
    values = np.asarray(values, np.float32)
    gate_scores = np.asarray(gate_scores, np.float32)
    W_q, b_q = np.asarray(W_q, np.float32), np.asarray(b_q, np.float32)
    W_k, b_k = np.asarray(W_k, np.float32), np.asarray(b_k, np.float32)
    W_v, b_v = np.asarray(W_v, np.float32), np.asarray(b_v, np.float32)
    W_o, b_o = np.asarray(W_o, np.float32), np.asarray(b_o, np.float32)
    assert queries.shape == (B, L_FULL, D_MODEL)

    if "nc" not in _NC_CACHE:
        _NC_CACHE["nc"] = build_core()
    nc = _NC_CACHE["nc"]
    in_maps = make_in_maps(queries, keys, values, gate_scores, W_q, b_q,
                           W_k, b_k, W_v, W_o)
    res = run_bass_kernel_spmd(nc, in_maps, core_ids=list(range(N_CORES)))

    HPC = H // 2
    attn = np.empty((B, H, L_FULL, S_FULL), np.float32)
    out = np.empty((B, L_FULL, D_MODEL), np.float32)
    corr = (W_o @ b_v + b_o).astype(np.float32)
    for b in range(B):
        r0 = res.results[b * 2]
        r1 = res.results[b * 2 + 1]
        attn[b, 0:HPC] = r0["attn"]
        attn[b, HPC:H] = r1["attn"]
        out[b] = r0["outp"] + r1["outp"] + corr
    return out, attn


# revision 25
# speedup vs baseline: 1.4812x; 1.0125x over previous
"""Trainium2 Bass kernel for nn_AttentionLayer_67817533604501.

Per-sample gated multi-head attention:
  Q = einsum('blm,bhkm,hkm->blhk', queries, gate, Wq) + bq   (same for K, V)
  attn = softmax(Q K^T / sqrt(dk)); out = (attn V) @ Wo^T + bo
Returns (out, attn) like the reference.

Sharding: 8 cores = 4 batches x 2 head-groups (8 heads each). Replicated
weights are sliced per head-group on host; per-core partial outputs of the
final projection are summed on host (exact: bias terms bo and the V-bias
contribution Wo@bv are added on host, mathematically identical because
attention rows sum to 1).

Device design (per core), all in transposed layouts so no on-device input
transposes are needed:
  QT,KT [feat, seq]; V [seq, feat(+ones col)] ; scores^T [s, l]
  exp on ScalarE reading PSUM; column sums via the ones column of V fused
  into the AV matmul; PE transposes of E produce attn[l, s]; the 1/colsum
  normalization is fused into the PSUM->SBUF copies (split DVE/ACT).
  Matmuls run in float32r (full PE rate, ~11-bit mantissa).
  Software pipeline: head h's transposes+normalize are interleaved into
  head h+1's scores/exp window so the PE never waits on ScalarE.
"""
import numpy as np
from contextlib import ExitStack

import concourse.bass as bass
import concourse.mybir as mybir
import concourse.tile as tile
from concourse import bacc
from concourse.bass_utils import run_bass_kernel_spmd
from concourse.masks import make_identity
from concourse.tile import add_dep_helper

F32 = mybir.dt.float32
F32R = mybir.dt.float32r
AF = mybir.ActivationFunctionType

B, L_FULL, S_FULL = 4, 1024, 1024
D_MODEL, H, DK = 1024, 16, 64
N_CORES = 8


def build_core(DM=1024, L=1024, S=1024, HPC=8, DKc=64, reps=1):
    """Build the per-core Bass module: one batch sample, HPC heads."""
    F = HPC * DKc
    MT, FT, LT, ST = DM // 128, F // 128, L // 128, S // 128
    LC, SC, DC = L // 512, S // 512, DM // 512
    assert LC >= 1 and SC >= 1 and DC >= 1 and FT >= 1 and ST % 4 == 0

    nc = bacc.Bacc("TRN2", target_bir_lowering=False, debug=False)

    xqT = nc.dram_tensor("xqT", [DM, L], F32R, kind="ExternalInput")
    xkT = nc.dram_tensor("xkT", [DM, S], F32R, kind="ExternalInput")
    xvT = nc.dram_tensor("xvT", [DM, S], F32R, kind="ExternalInput")
    gT = nc.dram_tensor("gT", [DM, F], F32, kind="ExternalInput")
    wqT = nc.dram_tensor("wqT", [DM, F], F32, kind="ExternalInput")
    wkT = nc.dram_tensor("wkT", [DM, F], F32, kind="ExternalInput")
    wvT = nc.dram_tensor("wvT", [DM, F], F32, kind="ExternalInput")
    woT = nc.dram_tensor("woT", [F, DM], F32R, kind="ExternalInput")
    bq = nc.dram_tensor("bq", [128, FT], F32, kind="ExternalInput")
    bk = nc.dram_tensor("bk", [128, FT], F32, kind="ExternalInput")
    attn_d = nc.dram_tensor("attn", [HPC, L, S], F32, kind="ExternalOutput")
    outp_d = nc.dram_tensor("outp", [L, DM], F32, kind="ExternalOutput")

    cfg = dict(DM=DM, L=L, S=S, HPC=HPC, DKc=DKc, F=F, MT=MT, FT=FT, LT=LT,
               ST=ST, LC=LC, SC=SC, DC=DC, xqT=xqT, xkT=xkT, xvT=xvT, gT=gT,
               wqT=wqT, wkT=wkT, wvT=wvT, woT=woT, bq=bq, bk=bk,
               attn_d=attn_d, outp_d=outp_d)
    with tile.TileContext(nc) as tc:
        for rep in range(reps):
            if rep:
                tc.strict_bb_all_engine_barrier()
            _emit_body(nc, tc, cfg, rep)
    nc.finalize()
    return nc


def _emit_body(nc, tc, v, rep=0):
    DM, L, S, HPC, DKc = v["DM"], v["L"], v["S"], v["HPC"], v["DKc"]
    F, MT, FT, LT, ST = v["F"], v["MT"], v["FT"], v["LT"], v["ST"]
    LC, SC, DC = v["LC"], v["SC"], v["DC"]
    xqT, xkT, xvT = v["xqT"], v["xkT"], v["xvT"]
    gT, wqT, wkT, wvT, woT = v["gT"], v["wqT"], v["wkT"], v["wvT"], v["woT"]
    bq, bk, attn_d, outp_d = v["bq"], v["bk"], v["attn_d"], v["outp_d"]
    MH = MT // 2

    with ExitStack() as ctx:
        pers = ctx.enter_context(tc.tile_pool(name="pers", bufs=1))
        QT = pers.tile([128, FT, L], F32R, tag="QT")
        KT = pers.tile([128, FT, S], F32R, tag="KT")
        Vt = pers.tile([128, ST, HPC, DKc + 1], F32R, tag="Vt")
        AVT = pers.tile([128, FT, L], F32R, tag="AVT")
        wo_sb = pers.tile([128, FT, DM], F32R, tag="wo")
        bq_sb = pers.tile([128, FT], F32, tag="bq")
        bk_sb = pers.tile([128, FT], F32, tag="bk")
        warm1 = pers.tile([128, 1], F32, tag="warm1")
        idf = pers.tile([128, 128], F32, tag="idf")
        idr = pers.tile([128, 128], F32R, tag="idr")
        ones1 = pers.tile([128, 1], F32, tag="ones1")

        # ---------------- projections ----------------
        with ExitStack() as pctx:
            gpool = pctx.enter_context(tc.tile_pool(name="gpool", bufs=1))
            wpool = pctx.enter_context(tc.tile_pool(name="wpool", bufs=1))
            wepool = pctx.enter_context(tc.tile_pool(name="wepool", bufs=2))
            xpool = pctx.enter_context(tc.tile_pool(name="xpool", bufs=3))
            pj = pctx.enter_context(tc.tile_pool(name="pj", bufs=4, space="PSUM"))

            def load_xh(xdram, lc, name, nch=2, eng=None):
                xh = xpool.tile([128, MT, 512], F32R, tag="xh", name=name)
                eng = eng or nc.sync
                mc = MT // nch
                for i in range(nch):
                    eng.dma_start(
                        out=xh[:, i * mc:(i + 1) * mc, :],
                        in_=xdram[i * (DM // nch):(i + 1) * (DM // nch),
                                  lc * 512:(lc + 1) * 512]
                        .rearrange("(mt p) l -> p mt l", p=128))
                return xh

            g_sb = gpool.tile([128, MT, F], F32, tag="g")

            def gate_weights(wdram, name, nch=2):
                w_sb = wpool.tile([128, MT, F], F32, tag="wraw", name=f"w_{name}")
                mc = MT // nch
                we = wepool.tile([128, MT, F], F32R, tag="weff",
                                 name=f"we_{name}")
                for i in range(nch):
                    nc.sync.dma_start(
                        w_sb[:, i * mc:(i + 1) * mc, :],
                        wdram[i * (DM // nch):(i + 1) * (DM // nch), :]
                        .rearrange("(mt p) f -> p mt f", p=128))
                    if name == "q":
                        # interleave gate chunk loads on the q (cold) path
                        nc.sync.dma_start(
                            g_sb[:, i * mc:(i + 1) * mc, :],
                            gT[i * (DM // nch):(i + 1) * (DM // nch), :]
                            .rearrange("(mt p) f -> p mt f", p=128))
                    sl = slice(i * mc, (i + 1) * mc)
                    nc.vector.tensor_mul(we[:, sl, :], w_sb[:, sl, :],
                                         g_sb[:, sl, :])
                return we

            def proj_qk(we, xhs, bias_sb, OT, pn):
                for lc, xh in enumerate(xhs):
                    for ft in range(FT):
                        ps = pj.tile([128, 512], F32, tag="pj",
                                     name=f"pj_{pn}{lc}_{ft}")
                        for mt in range(MT):
                            nc.tensor.matmul(
                                ps[:], we[:, mt, ft * 128:(ft + 1) * 128],
                                xh[:, mt, :],
                                start=(mt == 0), stop=(mt == MT - 1))
                        if (ft + lc) % 2 == 0:
                            nc.vector.tensor_scalar_add(
                                OT[:, ft, lc * 512:(lc + 1) * 512], ps[:],
                                bias_sb[:, ft:ft + 1])
                        else:
                            nc.scalar.activation(
                                OT[:, ft, lc * 512:(lc + 1) * 512], ps[:],
                                AF.Identity, bias=bias_sb[:, ft:ft + 1])

            # cold path: xq chunk 0 first, then quarter-chunked gate/wq
            xh_q = [load_xh(xqT, 0, "xh_q0", eng=nc.gpsimd)]
            we_q = gate_weights(wqT, "q", nch=4)
            if LC > 1:
                xh_q.append(load_xh(xqT, 1, "xh_q1", eng=nc.gpsimd))
            nc.sync.dma_start(bq_sb[:], bq[:])
            nc.sync.dma_start(bk_sb[:], bk[:])
            make_identity(nc, idf[:])
            nc.vector.tensor_copy(idr[:], idf[:])
            nc.vector.memset(ones1[:], 1.0)
            # pin the exp table set early so the first real exp doesn't stall
            nc.scalar.activation(warm1[:], ones1[:], AF.Exp)

            we_k = gate_weights(wkT, "k")
            xh_k = [load_xh(xkT, 0, "xh_k0")]
            proj_qk(we_q, xh_q, bq_sb, QT, "q")
            if SC > 1:
                xh_k.append(load_xh(xkT, 1, "xh_k1"))
            we_v = gate_weights(wvT, "v")
            xh_v = [load_xh(xvT, 0, "xh_v0")]
            proj_qk(we_k, xh_k, bk_sb, KT, "k")
            if SC > 1:
                xh_v.append(load_xh(xvT, 1, "xh_v1"))
            nc.sync.dma_start(
                wo_sb[:], woT[:].rearrange("(ft p) d -> p ft d", p=128))

            for sc in range(SC):
                xh = xh_v[sc]
                for stl in range(4):
                    st = sc * 4 + stl
                    ps = pj.tile([128, F], F32, tag="pj", name=f"pj_v{st}")
                    for mt in range(MT):
                        nc.tensor.matmul(
                            ps[:], xh[:, mt, stl * 128:(stl + 1) * 128],
                            we_v[:, mt, 0:F],
                            start=(mt == 0), stop=(mt == MT - 1))
                    dst = Vt[:, st, :, 0:DKc]
                    src2 = ps[:].rearrange("p (h k) -> p h k", k=DKc)
                    if st % 2 == 0:
                        nc.vector.tensor_copy(dst, src2)
                    else:
                        nc.scalar.copy(dst, src2)
            nc.vector.tensor_copy(
                Vt[:, :, :, DKc:DKc + 1],
                ones1[:, None, None, :].to_broadcast([128, ST, HPC, 1]))

        # ---------------- attention (head-pipelined; V proj inside h0) ------
        with ExitStack() as actx:
            epool = actx.enter_context(tc.tile_pool(name="epool", bufs=3))
            scp = actx.enter_context(tc.tile_pool(name="scp", bufs=2, space="PSUM"))
            avp = actx.enter_context(tc.tile_pool(name="avp", bufs=1, space="PSUM"))
            tpp = actx.enter_context(tc.tile_pool(name="tpp", bufs=2, space="PSUM"))
            apool = actx.enter_context(tc.tile_pool(name="apool", bufs=3))
            ipool = actx.enter_context(tc.tile_pool(name="ipool", bufs=2))
            dpool = actx.enter_context(tc.tile_pool(name="dpool", bufs=1, space="DRAM"))

            inv_d = dpool.tile([HPC, L], F32, tag="invd")
            NG = ST // 4
            groups = [(lt, g2) for lt in range(LT) for g2 in range(NG)]
            per_st = max(1, len(groups) // ST)

            def emit_group(pend, gi):
                h0 = pend["h"]
                lt, g2 = groups[gi]
                E0, ppi0 = pend["E"], pend["ppi"]
                if g2 == 0:
                    pend["astg"] = apool.tile([128, S], F32, tag="astg",
                                              name=f"astg{rep}_{h0}_{lt}")
                astg = pend["astg"]
                tpt = tpp.tile([128, 512], F32R, tag="tp",
                               name=f"tp{rep}_{h0}_{lt}_{g2}")
                for q in range(4):
                    st = g2 * 4 + q
                    nc.tensor.matmul(
                        tpt[:, q * 128:(q + 1) * 128],
                        E0[:, st, lt * 128:(lt + 1) * 128],
                        idr[:], is_transpose=True,
                        start=(q == 0), stop=(q == 3))
                sl = astg[:, g2 * 512:(g2 + 1) * 512]
                if gi % 4 < 3:
                    nc.vector.tensor_scalar_mul(sl, tpt[:], ppi0[:, lt:lt + 1])
                else:
                    nc.scalar.activation(sl, tpt[:], AF.Copy,
                                         scale=ppi0[:, lt:lt + 1])
                if g2 == NG - 1:
                    nc.sync.dma_start(
                        attn_d[h0, lt * 128:(lt + 1) * 128, :], astg[:])

            pend = None
            lazy_norm = {}

            def avt_slice(h0):
                return AVT[(h0 % 2) * 64:(h0 % 2) * 64 + 64, h0 // 2, :]

            def emit_avt_norm(h0):
                rep_t0 = lazy_norm.pop(h0, None)
                if rep_t0 is not None:
                    sl0 = avt_slice(h0)
                    pq0 = (h0 % 2) * 64
                    nc.vector.tensor_mul(sl0, sl0, rep_t0[pq0:pq0 + 64, :])

            for h in range(HPC):
                emit_avt_norm(h - 2)
                E_h = epool.tile([128, ST, L], F32R, tag="E", name=f"E{rep}_{h}")
                pq = (h % 2) * 64
                fth = h // 2
                av = avp.tile([65, L], F32, tag="av", name=f"av{rep}_{h}")

                def emit_av(st0):
                    for c in range(LC):
                        nc.tensor.matmul(
                            av[:, c * 512:(c + 1) * 512],
                            Vt[:, st0, h, :],
                            E_h[:, st0, c * 512:(c + 1) * 512],
                            start=(st0 == 0), stop=(st0 == ST - 1))

                gi = 0
                for st in range(ST):
                    ps = scp.tile([128, L], F32, tag="sc",
                                  name=f"sc{rep}_{h}_{st}")
                    for c in range(LC):
                        nc.tensor.matmul(
                            ps[:, c * 512:(c + 1) * 512],
                            KT[pq:pq + 64, fth, st * 128:(st + 1) * 128],
                            QT[pq:pq + 64, fth, c * 512:(c + 1) * 512],
                            start=True, stop=True)
                    nc.scalar.activation(E_h[:, st, :], ps[:], AF.Exp)
                    if st > 0:
                        emit_av(st - 1)
                    if pend is not None:
                        for _ in range(per_st):
                            if gi < len(groups):
                                emit_group(pend, gi)
                                gi += 1
                if pend is not None:
                    while gi < len(groups):
                        emit_group(pend, gi)
                        gi += 1
                emit_av(ST - 1)
                inv_row = ipool.tile([1, L], F32, tag="invr",
                                     name=f"invr{rep}_{h}")
                nc.vector.reciprocal(inv_row[:], av[64:65, :])
                d_out = nc.gpsimd.dma_start(out=inv_d[h], in_=inv_row[:])
                ppi = ipool.tile([128, LT], F32, tag="invpp",
                                 name=f"invpp{rep}_{h}")
                d_pp = nc.gpsimd.dma_start(
                    out=ppi[:], in_=inv_d[h].rearrange("(t p) -> p t", p=128))
                add_dep_helper(d_pp.ins, d_out.ins, reason="inv bounce RAW pp")
                rep_t = ipool.tile([128, L], F32, tag="invrep",
                                   name=f"invrep{rep}_{h}")
                src = inv_d[h]
                rap = bass.AP(tensor=src.tensor, offset=src.offset,
                              ap=[[0, 64], [1, L]])
                d_rep = nc.gpsimd.dma_start(out=rep_t[pq:pq + 64, :], in_=rap)
                add_dep_helper(d_rep.ins, d_out.ins, reason="inv bounce RAW rep")
                # stash unnormalized AVT now (frees the av PSUM slot);
                # multiply by 1/colsum lazily, two heads later
                nc.vector.tensor_copy(avt_slice(h), av[0:64, :])
                lazy_norm[h] = rep_t
                pend = {"h": h, "E": E_h, "ppi": ppi, "astg": None}

            gi = 0
            while gi < len(groups):
                emit_group(pend, gi)
                gi += 1
            emit_avt_norm(HPC - 2)
            emit_avt_norm(HPC - 1)

        # ---------------- output projection ----------------
        with ExitStack() as octx:
            opp = octx.enter_context(tc.tile_pool(name="opp", bufs=2, space="PSUM"))
            osb = octx.enter_context(tc.tile_pool(name="osb", bufs=3))
            for lt in range(LT):
                po = opp.tile([128, DM], F32, tag="op", name=f"op{rep}_{lt}")
                for dc in range(DC):
                    for jf in range(FT):
                        nc.tensor.matmul(
                            po[:, dc * 512:(dc + 1) * 512],
                            AVT[:, jf, lt * 128:(lt + 1) * 128],
                            wo_sb[:, jf, dc * 512:(dc + 1) * 512],
                            start=(jf == 0), stop=(jf == FT - 1))
                ob = osb.tile([128, DM], F32, tag="ob", name=f"ob{rep}_{lt}")
                if lt % 2:
                    nc.scalar.copy(ob[:], po[:])
                else:
                    nc.vector.tensor_copy(ob[:], po[:])
                nc.sync.dma_start(outp_d[lt * 128:(lt + 1) * 128, :], ob[:])


def make_in_maps(queries, keys, values, gate_scores, W_q, b_q, W_k, b_k,
                 W_v, W_o):
    """Host-side sharding + layout prep. Core c = b*2 + g."""
    HPC = H // 2
    F = HPC * DK
    in_maps = []
    c = np.ascontiguousarray
    for b in range(B):
        qT = c(queries[b].T)
        kT = c(keys[b].T)
        vT = c(values[b].T)
        for g in range(2):
            fsl = slice(g * F, (g + 1) * F)
            gTc = c(gate_scores[b, g * HPC:(g + 1) * HPC]
                    .reshape(F, D_MODEL).T)
            in_maps.append({
                "xqT": qT, "xkT": kT, "xvT": vT, "gT": gTc,
                "wqT": c((W_q[fsl] * 0.125).T),
                "wkT": c(W_k[fsl].T),
                "wvT": c(W_v[fsl].T),
                "woT": c(W_o[:, fsl].T),
                "bq": c((b_q[fsl] * 0.125).reshape(F // 128, 128).T),
                "bk": c(b_k[fsl].reshape(F // 128, 128).T),
            })
    return in_maps


_NC_CACHE = {}


def kernel(queries, keys, values, gate_scores, W_q, b_q, W_k, b_k, W_v, b_v,
           W_o, b_o):
    queries = np.asarray(queries, np.float32)
    keys = np.asarray(keys, np.float32)
    values = np.asarray(values, np.float32)
    gate_scores = np.asarray(gate_scores, np.float32)
    W_q, b_q = np.asarray(W_q, np.float32), np.asarray(b_q, np.float32)
    W_k, b_k = np.asarray(W_k, np.float32), np.asarray(b_k, np.float32)
    W_v, b_v = np.asarray(W_v, np.float32), np.asarray(b_v, np.float32)
    W_o, b_o = np.asarray(W_o, np.float32), np.asarray(b_o, np.float32)
    assert queries.shape == (B, L_FULL, D_MODEL)

    if "nc" not in _NC_CACHE:
        _NC_CACHE["nc"] = build_core()
    nc = _NC_CACHE["nc"]
    in_maps = make_in_maps(queries, keys, values, gate_scores, W_q, b_q,
                           W_k, b_k, W_v, W_o)
    res = run_bass_kernel_spmd(nc, in_maps, core_ids=list(range(N_CORES)))

    HPC = H // 2
    attn = np.empty((B, H, L_FULL, S_FULL), np.float32)
    out = np.empty((B, L_FULL, D_MODEL), np.float32)
    corr = (W_o @ b_v + b_o).astype(np.float32)
    for b in range(B):
        r0 = res.results[b * 2]
        r1 = res.results[b * 2 + 1]
        attn[b, 0:HPC] = r0["attn"]
        attn[b, HPC:H] = r1["attn"]
        out[b] = r0["outp"] + r1["outp"] + corr
    return out, attn


# revision 32
# speedup vs baseline: 1.4847x; 1.0023x over previous
"""Trainium2 Bass kernel for nn_AttentionLayer_67817533604501.

Per-sample gated multi-head attention:
  Q = einsum('blm,bhkm,hkm->blhk', queries, gate, Wq) + bq   (same for K, V)
  attn = softmax(Q K^T / sqrt(dk)); out = (attn V) @ Wo^T + bo
Returns (out, attn) like the reference.

Sharding: 8 cores = 4 batches x 2 head-groups (8 heads each). Replicated
weights are sliced per head-group on host; per-core partial outputs of the
final projection are summed on host (exact: bias terms bo and the V-bias
contribution Wo@bv are added on host, mathematically identical because
attention rows sum to 1).

Device design (per core), all in transposed layouts so no on-device input
transposes are needed:
  QT,KT [feat, seq]; V [seq, feat(+ones col)] ; scores^T [s, l]
  exp on ScalarE reading PSUM; column sums via the ones column of V fused
  into the AV matmul; PE transposes of E produce attn[l, s]; the 1/colsum
  normalization is fused into the PSUM->SBUF copies (split DVE/ACT).
  Matmuls run in float32r (full PE rate, ~11-bit mantissa).
  Software pipeline: head h's transposes+normalize are interleaved into
  head h+1's scores/exp window so the PE never waits on ScalarE.
"""
import numpy as np
from contextlib import ExitStack

import concourse.bass as bass
import concourse.mybir as mybir
import concourse.tile as tile
from concourse import bacc
from concourse.bass_utils import run_bass_kernel_spmd
from concourse.masks import make_identity
from concourse.tile import add_dep_helper

F32 = mybir.dt.float32
F32R = mybir.dt.float32r
AF = mybir.ActivationFunctionType

B, L_FULL, S_FULL = 4, 1024, 1024
D_MODEL, H, DK = 1024, 16, 64
N_CORES = 8


def build_core(DM=1024, L=1024, S=1024, HPC=8, DKc=64, reps=1):
    """Build the per-core Bass module: one batch sample, HPC heads."""
    F = HPC * DKc
    MT, FT, LT, ST = DM // 128, F // 128, L // 128, S // 128
    LC, SC, DC = L // 512, S // 512, DM // 512
    assert LC >= 1 and SC >= 1 and DC >= 1 and FT >= 1 and ST % 4 == 0

    nc = bacc.Bacc("TRN2", target_bir_lowering=False, debug=False)

    xqT = nc.dram_tensor("xqT", [DM, L], F32R, kind="ExternalInput")
    xkT = nc.dram_tensor("xkT", [DM, S], F32R, kind="ExternalInput")
    xvT = nc.dram_tensor("xvT", [DM, S], F32R, kind="ExternalInput")
    gT = nc.dram_tensor("gT", [DM, F], F32, kind="ExternalInput")
    wqT = nc.dram_tensor("wqT", [DM, F], F32, kind="ExternalInput")
    wkT = nc.dram_tensor("wkT", [DM, F], F32, kind="ExternalInput")
    wvT = nc.dram_tensor("wvT", [DM, F], F32, kind="ExternalInput")
    woT = nc.dram_tensor("woT", [F, DM], F32R, kind="ExternalInput")
    bq = nc.dram_tensor("bq", [128, FT], F32, kind="ExternalInput")
    bk = nc.dram_tensor("bk", [128, FT], F32, kind="ExternalInput")
    attn_d = nc.dram_tensor("attn", [HPC, L, S], F32, kind="ExternalOutput")
    outp_d = nc.dram_tensor("outp", [L, DM], F32, kind="ExternalOutput")

    cfg = dict(DM=DM, L=L, S=S, HPC=HPC, DKc=DKc, F=F, MT=MT, FT=FT, LT=LT,
               ST=ST, LC=LC, SC=SC, DC=DC, xqT=xqT, xkT=xkT, xvT=xvT, gT=gT,
               wqT=wqT, wkT=wkT, wvT=wvT, woT=woT, bq=bq, bk=bk,
               attn_d=attn_d, outp_d=outp_d)
    with tile.TileContext(nc) as tc:
        for rep in range(reps):
            if rep:
                tc.strict_bb_all_engine_barrier()
            _emit_body(nc, tc, cfg, rep)
    nc.finalize()
    return nc


def _emit_body(nc, tc, v, rep=0):
    DM, L, S, HPC, DKc = v["DM"], v["L"], v["S"], v["HPC"], v["DKc"]
    F, MT, FT, LT, ST = v["F"], v["MT"], v["FT"], v["LT"], v["ST"]
    LC, SC, DC = v["LC"], v["SC"], v["DC"]
    xqT, xkT, xvT = v["xqT"], v["xkT"], v["xvT"]
    gT, wqT, wkT, wvT, woT = v["gT"], v["wqT"], v["wkT"], v["wvT"], v["woT"]
    bq, bk, attn_d, outp_d = v["bq"], v["bk"], v["attn_d"], v["outp_d"]
    MH = MT // 2

    with ExitStack() as ctx:
        pers = ctx.enter_context(tc.tile_pool(name="pers", bufs=1))
        QT = pers.tile([128, FT, L], F32R, tag="QT")
        KT = pers.tile([128, FT, S], F32R, tag="KT")
        Vt = pers.tile([128, ST, HPC, DKc + 1], F32R, tag="Vt")
        AVT = pers.tile([128, FT, L], F32R, tag="AVT")
        wo_sb = pers.tile([128, FT, DM], F32R, tag="wo")
        bq_sb = pers.tile([128, FT], F32, tag="bq")
        bk_sb = pers.tile([128, FT], F32, tag="bk")
        warm1 = pers.tile([128, 1], F32, tag="warm1")
        idf = pers.tile([128, 128], F32, tag="idf")
        idr = pers.tile([128, 128], F32R, tag="idr")
        ones1 = pers.tile([128, 1], F32, tag="ones1")

        # ---------------- projections ----------------
        with ExitStack() as pctx:
            gpool = pctx.enter_context(tc.tile_pool(name="gpool", bufs=1))
            wpool = pctx.enter_context(tc.tile_pool(name="wpool", bufs=1))
            wepool = pctx.enter_context(tc.tile_pool(name="wepool", bufs=2))
            xpool = pctx.enter_context(tc.tile_pool(name="xpool", bufs=3))
            pj = pctx.enter_context(tc.tile_pool(name="pj", bufs=4, space="PSUM"))

            def load_xh(xdram, lc, name, nch=2, eng=None):
                xh = xpool.tile([128, MT, 512], F32R, tag="xh", name=name)
                eng = eng or nc.sync
                mc = MT // nch
                for i in range(nch):
                    eng.dma_start(
                        out=xh[:, i * mc:(i + 1) * mc, :],
                        in_=xdram[i * (DM // nch):(i + 1) * (DM // nch),
                                  lc * 512:(lc + 1) * 512]
                        .rearrange("(mt p) l -> p mt l", p=128))
                return xh

            g_sb = gpool.tile([128, MT, F], F32, tag="g")

            def gate_weights(wdram, name, nch=2):
                w_sb = wpool.tile([128, MT, F], F32, tag="wraw", name=f"w_{name}")
                mc = MT // nch
                we = wepool.tile([128, MT, F], F32R, tag="weff",
                                 name=f"we_{name}")
                for i in range(nch):
                    nc.sync.dma_start(
                        w_sb[:, i * mc:(i + 1) * mc, :],
                        wdram[i * (DM // nch):(i + 1) * (DM // nch), :]
                        .rearrange("(mt p) f -> p mt f", p=128))
                    if name == "q":
                        # interleave gate chunk loads on the q (cold) path
                        nc.sync.dma_start(
                            g_sb[:, i * mc:(i + 1) * mc, :],
                            gT[i * (DM // nch):(i + 1) * (DM // nch), :]
                            .rearrange("(mt p) f -> p mt f", p=128))
                    sl = slice(i * mc, (i + 1) * mc)
                    nc.vector.tensor_mul(we[:, sl, :], w_sb[:, sl, :],
                                         g_sb[:, sl, :])
                return we

            def proj_qk(we, xhs, bias_sb, OT, pn):
                for lc, xh in enumerate(xhs):
                    for ft in range(FT):
                        ps = pj.tile([128, 512], F32, tag="pj",
                                     name=f"pj_{pn}{lc}_{ft}")
                        for mt in range(MT):
                            nc.tensor.matmul(
                                ps[:], we[:, mt, ft * 128:(ft + 1) * 128],
                                xh[:, mt, :],
                                start=(mt == 0), stop=(mt == MT - 1))
                        if (ft + lc) % 2 == 0:
                            nc.vector.tensor_scalar_add(
                                OT[:, ft, lc * 512:(lc + 1) * 512], ps[:],
                                bias_sb[:, ft:ft + 1])
                        else:
                            nc.scalar.activation(
                                OT[:, ft, lc * 512:(lc + 1) * 512], ps[:],
                                AF.Identity, bias=bias_sb[:, ft:ft + 1])

            # cold path: xq chunk 0 first, then quarter-chunked gate/wq
            xh_q = [load_xh(xqT, 0, "xh_q0", eng=nc.gpsimd)]
            we_q = gate_weights(wqT, "q", nch=4)
            if LC > 1:
                xh_q.append(load_xh(xqT, 1, "xh_q1", eng=nc.gpsimd))
            nc.gpsimd.dma_start(out=bq_sb[:], in_=bq[:])
            nc.gpsimd.dma_start(out=bk_sb[:], in_=bk[:])
            make_identity(nc, idf[:])
            nc.vector.tensor_copy(idr[:], idf[:])
            nc.vector.memset(ones1[:], 1.0)
            # pin the exp table set early so the first real exp doesn't stall
            nc.scalar.activation(warm1[:], ones1[:], AF.Exp)

            we_k = gate_weights(wkT, "k")
            xh_k = [load_xh(xkT, 0, "xh_k0")]
            proj_qk(we_q, xh_q, bq_sb, QT, "q")
            if SC > 1:
                xh_k.append(load_xh(xkT, 1, "xh_k1"))
            we_v = gate_weights(wvT, "v")
            xh_v = [load_xh(xvT, 0, "xh_v0")]
            proj_qk(we_k, xh_k, bk_sb, KT, "k")
            if SC > 1:
                xh_v.append(load_xh(xvT, 1, "xh_v1"))
            nc.sync.dma_start(
                wo_sb[:], woT[:].rearrange("(ft p) d -> p ft d", p=128))

            for sc in range(SC):
                xh = xh_v[sc]
                for stl in range(4):
                    st = sc * 4 + stl
                    ps = pj.tile([128, F], F32, tag="pj", name=f"pj_v{st}")
                    for mt in range(MT):
                        nc.tensor.matmul(
                            ps[:], xh[:, mt, stl * 128:(stl + 1) * 128],
                            we_v[:, mt, 0:F],
                            start=(mt == 0), stop=(mt == MT - 1))
                    dst = Vt[:, st, :, 0:DKc]
                    src2 = ps[:].rearrange("p (h k) -> p h k", k=DKc)
                    if st % 2 == 0:
                        nc.vector.tensor_copy(dst, src2)
                    else:
                        nc.scalar.copy(dst, src2)
            nc.vector.tensor_copy(
                Vt[:, :, :, DKc:DKc + 1],
                ones1[:, None, None, :].to_broadcast([128, ST, HPC, 1]))

        # ---------------- attention (head-pipelined; V proj inside h0) ------
        with ExitStack() as actx:
            epool = actx.enter_context(tc.tile_pool(name="epool", bufs=3))
            scp = actx.enter_context(tc.tile_pool(name="scp", bufs=2, space="PSUM"))
            avp = actx.enter_context(tc.tile_pool(name="avp", bufs=1, space="PSUM"))
            tpp = actx.enter_context(tc.tile_pool(name="tpp", bufs=2, space="PSUM"))
            apool = actx.enter_context(tc.tile_pool(name="apool", bufs=3))
            ipool = actx.enter_context(tc.tile_pool(name="ipool", bufs=2))
            dpool = actx.enter_context(tc.tile_pool(name="dpool", bufs=1, space="DRAM"))

            inv_d = dpool.tile([HPC, L], F32, tag="invd")
            NG = ST // 4
            groups = [(lt, g2) for lt in range(LT) for g2 in range(NG)]
            per_st = max(1, len(groups) // ST)

            def emit_group(pend, gi):
                h0 = pend["h"]
                lt, g2 = groups[gi]
                E0, ppi0 = pend["E"], pend["ppi"]
                if g2 == 0:
                    pend["astg"] = apool.tile([128, S], F32, tag="astg",
                                              name=f"astg{rep}_{h0}_{lt}")
                astg = pend["astg"]
                tpt = tpp.tile([128, 512], F32R, tag="tp",
                               name=f"tp{rep}_{h0}_{lt}_{g2}")
                for q in range(4):
                    st = g2 * 4 + q
                    nc.tensor.matmul(
                        tpt[:, q * 128:(q + 1) * 128],
                        E0[:, st, lt * 128:(lt + 1) * 128],
                        idr[:], is_transpose=True,
                        start=(q == 0), stop=(q == 3))
                sl = astg[:, g2 * 512:(g2 + 1) * 512]
                if gi % 4 < 3:
                    nc.vector.tensor_scalar_mul(sl, tpt[:], ppi0[:, lt:lt + 1])
                else:
                    nc.scalar.activation(sl, tpt[:], AF.Copy,
                                         scale=ppi0[:, lt:lt + 1])
                if g2 == NG - 1:
                    nc.sync.dma_start(
                        attn_d[h0, lt * 128:(lt + 1) * 128, :], astg[:])

            pend = None
            lazy_norm = {}

            def avt_slice(h0):
                return AVT[(h0 % 2) * 64:(h0 % 2) * 64 + 64, h0 // 2, :]

            def emit_avt_norm(h0):
                rep_t0 = lazy_norm.pop(h0, None)
                if rep_t0 is not None:
                    sl0 = avt_slice(h0)
                    pq0 = (h0 % 2) * 64
                    nc.vector.tensor_mul(sl0, sl0, rep_t0[pq0:pq0 + 64, :])

            for h in range(HPC):
                emit_avt_norm(h - 2)
                E_h = epool.tile([128, ST, L], F32R, tag="E", name=f"E{rep}_{h}")
                pq = (h % 2) * 64
                fth = h // 2
                av = avp.tile([65, L], F32, tag="av", name=f"av{rep}_{h}")

                def emit_av(st0):
                    for c in range(LC):
                        nc.tensor.matmul(
                            av[:, c * 512:(c + 1) * 512],
                            Vt[:, st0, h, :],
                            E_h[:, st0, c * 512:(c + 1) * 512],
                            start=(st0 == 0), stop=(st0 == ST - 1))

                gi = 0
                for st in range(ST):
                    ps = scp.tile([128, L], F32, tag="sc",
                                  name=f"sc{rep}_{h}_{st}")
                    for c in range(LC):
                        nc.tensor.matmul(
                            ps[:, c * 512:(c + 1) * 512],
                            KT[pq:pq + 64, fth, st * 128:(st + 1) * 128],
                            QT[pq:pq + 64, fth, c * 512:(c + 1) * 512],
                            start=True, stop=True)
                    nc.scalar.activation(E_h[:, st, :], ps[:], AF.Exp)
                    if st > 0:
                        emit_av(st - 1)
                    if pend is not None:
                        for _ in range(per_st):
                            if gi < len(groups):
                                emit_group(pend, gi)
                                gi += 1
                if pend is not None:
                    while gi < len(groups):
                        emit_group(pend, gi)
                        gi += 1
                emit_av(ST - 1)
                inv_row = ipool.tile([1, L], F32, tag="invr",
                                     name=f"invr{rep}_{h}")
                nc.vector.reciprocal(inv_row[:], av[64:65, :])
                d_out = nc.gpsimd.dma_start(out=inv_d[h], in_=inv_row[:])
                ppi = ipool.tile([128, LT], F32, tag="invpp",
                                 name=f"invpp{rep}_{h}")
                d_pp = nc.gpsimd.dma_start(
                    out=ppi[:], in_=inv_d[h].rearrange("(t p) -> p t", p=128))
                add_dep_helper(d_pp.ins, d_out.ins, reason="inv bounce RAW pp")
                rep_t = ipool.tile([128, L], F32, tag="invrep",
                                   name=f"invrep{rep}_{h}")
                src = inv_d[h]
                rap = bass.AP(tensor=src.tensor, offset=src.offset,
                              ap=[[0, 64], [1, L]])
                d_rep = nc.gpsimd.dma_start(out=rep_t[pq:pq + 64, :], in_=rap)
                add_dep_helper(d_rep.ins, d_out.ins, reason="inv bounce RAW rep")
                # stash unnormalized AVT now (frees the av PSUM slot);
                # multiply by 1/colsum lazily, two heads later
                nc.vector.tensor_copy(avt_slice(h), av[0:64, :])
                lazy_norm[h] = rep_t
                pend = {"h": h, "E": E_h, "ppi": ppi, "astg": None}

            gi = 0
            while gi < len(groups):
                emit_group(pend, gi)
                gi += 1
            emit_avt_norm(HPC - 2)
            emit_avt_norm(HPC - 1)

        # ---------------- output projection ----------------
        with ExitStack() as octx:
            opp = octx.enter_context(tc.tile_pool(name="opp", bufs=2, space="PSUM"))
            osb = octx.enter_context(tc.tile_pool(name="osb", bufs=3))
            for lt in range(LT):
                po = opp.tile([128, DM], F32, tag="op", name=f"op{rep}_{lt}")
                for dc in range(DC):
                    for jf in range(FT):
                        nc.tensor.matmul(
                            po[:, dc * 512:(dc + 1) * 512],
                            AVT[:, jf, lt * 128:(lt + 1) * 128],
                            wo_sb[:, jf, dc * 512:(dc + 1) * 512],
                            start=(jf == 0), stop=(jf == FT - 1))
                ob = osb.tile([128, DM], F32, tag="ob", name=f"ob{rep}_{lt}")
                if lt % 2:
                    nc.scalar.copy(ob[:], po[:])
                else:
                    nc.vector.tensor_copy(ob[:], po[:])
                nc.sync.dma_start(outp_d[lt * 128:(lt + 1) * 128, :], ob[:])


def make_in_maps(queries, keys, values, gate_scores, W_q, b_q, W_k, b_k,
                 W_v, W_o):
    """Host-side sharding + layout prep. Core c = b*2 + g."""
    HPC = H // 2
    F = HPC * DK
    in_maps = []
    c = np.ascontiguousarray
    for b in range(B):
        qT = c(queries[b].T)
        kT = c(keys[b].T)
        vT = c(values[b].T)
        for g in range(2):
            fsl = slice(g * F, (g + 1) * F)
            gTc = c(gate_scores[b, g * HPC:(g + 1) * HPC]
                    .reshape(F, D_MODEL).T)
            in_maps.append({
                "xqT": qT, "xkT": kT, "xvT": vT, "gT": gTc,
                "wqT": c((W_q[fsl] * 0.125).T),
                "wkT": c(W_k[fsl].T),
                "wvT": c(W_v[fsl].T),
                "woT": c(W_o[:, fsl].T),
                "bq": c((b_q[fsl] * 0.125).reshape(F // 128, 128).T),
                "bk": c(b_k[fsl].reshape(F // 128, 128).T),
            })
    return in_maps


_NC_CACHE = {}


def kernel(queries, keys, values, gate_scores, W_q, b_q, W_k, b_k, W_v, b_v,
           W_o, b_o):
    queries = np.asarray(queries, np.float32)
    keys = np.asarray(keys, np.float32)
    values = np.asarray(values, np.float32)
    gate_scores = np.asarray(gate_scores, np.float32)
    W_q, b_q = np.asarray(W_q, np.float32), np.asarray(b_q, np.float32)
    W_k, b_k = np.asarray(W_k, np.float32), np.asarray(b_k, np.float32)
    W_v, b_v = np.asarray(W_v, np.float32), np.asarray(b_v, np.float32)
    W_o, b_o = np.asarray(W_o, np.float32), np.asarray(b_o, np.float32)
    assert queries.shape == (B, L_FULL, D_MODEL)

    if "nc" not in _NC_CACHE:
        _NC_CACHE["nc"] = build_core()
    nc = _NC_CACHE["nc"]
    in_maps = make_in_maps(queries, keys, values, gate_scores, W_q, b_q,
                           W_k, b_k, W_v, W_o)
    res = run_bass_kernel_spmd(nc, in_maps, core_ids=list(range(N_CORES)))

    HPC = H // 2
    attn = np.empty((B, H, L_FULL, S_FULL), np.float32)
    out = np.empty((B, L_FULL, D_MODEL), np.float32)
    corr = (W_o @ b_v + b_o).astype(np.float32)
    for b in range(B):
        r0 = res.results[b * 2]
        r1 = res.results[b * 2 + 1]
        attn[b, 0:HPC] = r0["attn"]
        attn[b, HPC:H] = r1["attn"]
        out[b] = r0["outp"] + r1["outp"] + corr
    return out, attn
